# revision 33
# baseline (speedup 1.0000x reference)
"""Trainium2 Bass kernel for a dense transformer encoder block.

Sharding: pure data-parallel, zero collectives. 8 cores; core c handles
batch b = c//2, query rows half = c%2 (1024 of 2048 seq positions).
Each core receives the full (sequence-rotated) x[b]^T so it can compute
K/V over all 2048 keys locally; queries are always columns 0:1024 of the
rotated x^T (attention is permutation-invariant over the key axis).

v3: query-chunk-outer attention with software-pipelined overlap: during
the second query-chunk's attention pass (which is paced by the Scalar
engine's softmax exps), the PE queue is fed the first chunk's wo
projection + LN1 + transposes and its full w1 FFN layer, one work item
per kt-pair slot.  fp8e4 DoubleRow matmuls (2 k-tiles per instruction)
for the V/Q/K projections and the attention*V matmul; fp8 scores
operands and fp8 ctxT/wo.  All fp8 prescales are powers of two and are
compensated exactly:
  - wq,wk x32 -> qh/kh store 32q/32k in fp8; scores psum = 1024*(q.k);
    exp runs with scale=2^-13 and bias=-ln16 (pt = p/16, max ~42 << 240).
  - wv x32, ones-column of V' = 1.0 -> ctx psum rows = 2*sum(p)v, denom
    row = sum(p)/16; reciprocal-normalize yields ctxT = 32*ctx in fp8.
  - wo x64 in fp8 -> wo psum = 2048*attn_out; one fused DVE op computes
    psum*2^-11 + x residual.
Scores run K=128 against zero-padded kh tiles (no PE tiling-mode
switches, keeps ctx DoubleRow weight-loads hidden).  w1 streams in
double-buffered chunks; w2 is resident only for the tail.
"""

import sys

if "/opt/trn_rl_repo" not in sys.path:
    sys.path.insert(0, "/opt/trn_rl_repo")

import numpy as np

B, S, D, H, DK, DFF = 4, 2048, 768, 12, 64, 3072
NCORES = 8
QR = 1024  # query rows per core
EPS = 1e-6
P = 128
NE = D // P  # 6 e-tiles (contraction over model dim)
NEP = NE // 2  # 3 DoubleRow pairs
NS = S // P  # 16 s-tiles (key positions)
NSP = NS // 2  # 8 DoubleRow kt pairs
NQ = QR // P  # 8 q-tiles
NF = DFF // P  # 24 f-tiles
HW = 80  # padded per-head width in V' (64 d + ones col + pad)
LN16 = 2.772588722239781  # ln(16)

_CACHE = {}


def _build(skip_affine):
    from contextlib import ExitStack

    import concourse.bass as bass
    import concourse.tile as tile
    from concourse import bacc, mybir
    from concourse.masks import make_identity

    dt = mybir.dt
    f32 = dt.float32
    bf16 = dt.bfloat16
    fp8 = dt.float8e4
    AF = mybir.ActivationFunctionType
    OP = mybir.AluOpType
    DR = mybir.MatmulPerfMode.DoubleRow

    nc = bacc.Bacc("TRN2", target_bir_lowering=False, debug=False)

    xt_d = nc.dram_tensor("xt", [D, S], fp8, kind="ExternalInput")
    xh_d = nc.dram_tensor("xh", [QR, D], bf16, kind="ExternalInput")
    wq_d = nc.dram_tensor("wq", [D, D], fp8, kind="ExternalInput")  # x32
    wk_d = nc.dram_tensor("wk", [D, D], fp8, kind="ExternalInput")  # x32
    wv_d = nc.dram_tensor("wv", [D, D], fp8, kind="ExternalInput")  # x32
    wo_d = nc.dram_tensor("wo", [D, D], fp8, kind="ExternalInput")  # x64
    w1_d = nc.dram_tensor("w1", [D, DFF], bf16, kind="ExternalInput")
    w2_d = nc.dram_tensor("w2", [DFF, D], bf16, kind="ExternalInput")
    b1_d = nc.dram_tensor("b1t", [P, NF], f32, kind="ExternalInput")  # b1 T'd
    b2_d = nc.dram_tensor("b2r", [1, D], bf16, kind="ExternalInput")
    ln1a_d = nc.dram_tensor("ln1a", [P, D], f32, kind="ExternalInput")  # bcast
    ln1b_d = nc.dram_tensor("ln1b", [P, D], f32, kind="ExternalInput")
    ln2a_d = nc.dram_tensor("ln2a", [P, D], f32, kind="ExternalInput")
    ln2b_d = nc.dram_tensor("ln2b", [P, D], f32, kind="ExternalInput")
    out_d = nc.dram_tensor("out", [QR, D], f32, kind="ExternalOutput")

    def dram3(d_ap, p=P):
        return d_ap.rearrange("(n p) s -> p n s", p=p)

    with tile.TileContext(nc) as tc:
        with ExitStack() as ctx:
            const = ctx.enter_context(tc.tile_pool(name="const", bufs=1))
            ones_bf = const.tile([1, P], bf16)
            nc.gpsimd.memset(ones_bf[:], 1.0)
            ident = const.tile([P, P], f32)
            make_identity(nc, ident[:])
            b1_sb = const.tile([P, NF], f32)
            nc.sync.dma_start(b1_sb[:], b1_d.ap())
            b2_sb = const.tile([1, D], bf16)
            nc.sync.dma_start(b2_sb[:], b2_d.ap())
            expb = const.tile([P, 1], f32)  # exp bias: -ln(16)
            nc.gpsimd.memset(expb[:], -LN16)

            # persistent (whole kernel) left-side pools
            xp = ctx.enter_context(tc.tile_pool(name="xp", bufs=1))
            x1 = xp.tile([P, NQ, D], f32, tag="x1")
            x1t = xp.tile([P, NE, QR], bf16, tag="x1t")
            xhp = ctx.enter_context(tc.tile_pool(name="xhp", bufs=1))
            spL = ctx.enter_context(tc.tile_pool(name="spL", bufs=1))
            lnc = ctx.enter_context(tc.tile_pool(name="lnc", bufs=1))
            if skip_affine:
                l1a = l1b = l2a = l2b = None
            else:
                l1a = lnc.tile([P, D], f32, tag="l1a")
                l1b = lnc.tile([P, D], f32, tag="l1b")
                l2a = lnc.tile([P, D], f32, tag="l2a")
                l2b = lnc.tile([P, D], f32, tag="l2b")
                nc.sync.dma_start(l1a[:], ln1a_d.ap())
                nc.sync.dma_start(l1b[:], ln1b_d.ap())
                nc.sync.dma_start(l2a[:], ln2a_d.ap())
                nc.sync.dma_start(l2b[:], ln2b_d.ap())
            w1p = ctx.enter_context(tc.tile_pool(name="w1p", bufs=2))
            # bufs=2: ht(qc1) relu must not wait on ht(qc0)'s w2 readers,
            # which sit later in the PE queue (would deadlock at bufs=1)
            htp = ctx.enter_context(tc.tile_pool(name="htp", bufs=2))

            # right side: ctx^T (fp8, = 32*ctx) + wo (fp8, = 64*wo)
            ctp = ctx.enter_context(tc.tile_pool(name="ctp", bufs=1, side="right"))
            ctxT = ctp.tile([P, NE, QR], fp8, tag="ctxT")
            wo_sb = ctp.tile([P, NE, D], fp8, tag="wo")

            psX = ctx.enter_context(tc.tile_pool(name="psX", bufs=2, space="PSUM"))

            es_ab = ExitStack()  # attention-lifetime pools
            kqv = es_ab.enter_context(tc.tile_pool(name="kqv", bufs=1))
            vo = kqv.tile([P, NSP, 2, H * HW], fp8, tag="vo")
            vo5 = vo[:, :, :, :].rearrange("p s j (h w) -> p s j h w", w=HW)
            kqr = es_ab.enter_context(tc.tile_pool(name="kqr", bufs=6))
            ptp = es_ab.enter_context(tc.tile_pool(name="ptp", bufs=2))
            up = es_ab.enter_context(tc.tile_pool(name="up", bufs=2))
            psS = es_ab.enter_context(tc.tile_pool(name="psS", bufs=2, space="PSUM"))
            psC = es_ab.enter_context(tc.tile_pool(name="psC", bufs=1, space="PSUM"))

            def layer_norm(tin, out_ap, a_bc, b_bc, spool):
                st6 = spool.tile([P, 2, 6], f32, tag="st6")
                nc.vector.bn_stats(st6[:, 0, :], tin[:, 0:384])
                nc.vector.bn_stats(st6[:, 1, :], tin[:, 384:768])
                mv = spool.tile([P, 2], f32, tag="mv")
                nc.vector.bn_aggr(mv[:], st6[:])
                std = spool.tile([P, 1], f32, tag="std")
                nc.scalar.activation(
                    std[:], mv[:, 1:2], AF.Sqrt, scale=float(D) / (D - 1)
                )
                stde = spool.tile([P, 1], f32, tag="stde")
                nc.vector.tensor_scalar_add(stde[:], std[:], EPS)
                rstd = spool.tile([P, 1], f32, tag="rstd")
                nc.vector.reciprocal(rstd[:], stde[:])
                if skip_affine:
                    nc.vector.tensor_scalar(
                        out_ap, tin[:], mv[:, 0:1], rstd[:],
                        op0=OP.subtract, op1=OP.mult,
                    )
                else:
                    yc = spool.tile([P, D], f32, tag="yc")
                    nc.vector.tensor_scalar(
                        yc[:], tin[:], mv[:, 0:1], rstd[:],
                        op0=OP.subtract, op1=OP.mult,
                    )
                    y2 = spool.tile([P, D], f32, tag="y2")
                    nc.vector.tensor_tensor(y2[:], yc[:], a_bc, OP.mult)
                    nc.vector.tensor_tensor(out_ap, y2[:], b_bc, OP.add)

            w1tiles = {}

            def w1_dma(qc, c):
                t = w1p.tile([P, NE, 4 * P], bf16, tag="w1c")
                for et in range(NE):
                    nc.sync.dma_start(
                        t[:, et, :],
                        w1_d.ap()[
                            et * P : (et + 1) * P, c * 4 * P : (c + 1) * 4 * P
                        ],
                    )
                w1tiles[(qc, c)] = t

            ht_t = [None, None]

            def w1_item(qc, f_t):
                c = f_t // 4
                if f_t % 4 == 0 and c + 2 < NF // 4:
                    w1_dma(qc, c + 2)
                if f_t == 0:
                    ht_new = htp.tile([P, NF, 512], bf16, tag="ht")
                    ht_t[qc] = ht_new
                ht = ht_t[qc]
                w1c = w1tiles[(qc, c)]
                ps = psX.tile([P, 512], f32, tag="x")
                for et in range(NE):
                    nc.tensor.matmul(
                        ps[:],
                        w1c[:, et, (f_t % 4) * P : (f_t % 4 + 1) * P],
                        x1t[:, et, qc * 512 : (qc + 1) * 512],
                        start=(et == 0),
                        stop=(et == NE - 1),
                    )
                nc.scalar.activation(
                    ht[:, f_t, :], ps[:], AF.Relu, bias=b1_sb[:, f_t : f_t + 1]
                )

            def c_item(qt_i, xh_sb, xo):
                # wo projection (fp8: psum = 2048*attn) + residual + LN1 + x1^T
                tsb = spL.tile([P, D], f32, tag="tsb")
                for dc, cw in ((0, 512), (512, 256)):
                    ps = psX.tile([P, 512], f32, tag="x")
                    for dt_i in range(NE):
                        nc.tensor.matmul(
                            ps[:, 0:cw],
                            ctxT[:, dt_i, qt_i * P : (qt_i + 1) * P],
                            wo_sb[:, dt_i, dc : dc + cw],
                            start=(dt_i == 0),
                            stop=(dt_i == NE - 1),
                        )
                    nc.vector.scalar_tensor_tensor(
                        tsb[:, dc : dc + cw],
                        ps[:, 0:cw],
                        float(2.0 ** -11),
                        xh_sb[:, qt_i - xo, dc : dc + cw],
                        op0=OP.mult,
                        op1=OP.add,
                    )
                layer_norm(tsb[:], x1[:, qt_i, :], l1a, l1b, spL)
                for dt_i in range(NE):
                    pst = psX.tile([P, 512], f32, tag="x")
                    nc.tensor.transpose(
                        pst[:, 0:P], x1[:, qt_i, dt_i * P : (dt_i + 1) * P],
                        ident[:],
                    )
                    nc.vector.tensor_copy(
                        x1t[:, dt_i, qt_i * P : (qt_i + 1) * P], pst[:, 0:P]
                    )

            qh_t = [None] * (H // 2)
            kh_t = [None] * (H // 2)
            pcs = [None, None]

            def kt_body(hp, qc, ktp):
                qh = qh_t[hp]
                kh0, kh1 = kh_t[hp]
                pc0, pc1 = pcs
                pt = ptp.tile([P, 2, 1024], fp8, tag="pt")
                for j in range(2):
                    kt_i = 2 * ktp + j
                    ps = psS.tile([P, 1024], f32, tag="psS")
                    for hh, khp in ((0, kh0), (1, kh1)):
                        nc.tensor.matmul(
                            ps[:, hh * 512 : hh * 512 + 512],
                            khp[:, kt_i * P : (kt_i + 1) * P],
                            qh[:, qc * 512 : (qc + 1) * 512],
                            start=True,
                            stop=True,
                        )
                    # pt = exp(scores)/16; scores psum = 1024*(q.k)
                    nc.scalar.activation(
                        pt[:, j, :], ps[:], AF.Exp,
                        bias=expb[:], scale=float(2.0 ** -13),
                    )
                for hh, pc in ((0, pc0), (1, pc1)):
                    h = 2 * hp + hh
                    nc.tensor.matmul(
                        pc[:],
                        vo5[:, ktp, :, h, 0 : DK + 1],
                        pt[:, :, hh * 512 : hh * 512 + 512],
                        start=(ktp == 0),
                        stop=(ktp == NSP - 1),
                        perf_mode=DR,
                    )

            def chain(hp, qc):
                # softmax normalize: ctxT = 32*ctx in fp8
                pc0, pc1 = pcs
                for hh, pc in ((0, pc0), (1, pc1)):
                    dcp = up.tile([1, 512], f32, tag="dcp")
                    nc.vector.tensor_copy(dcp[:], pc[DK : DK + 1, :])
                    rcp = up.tile([1, 512], f32, tag="rcp")
                    nc.vector.reciprocal_approx_fast(rcp[:], dcp[:])
                    rb = up.tile([DK, 512], f32, tag="rb")
                    nc.gpsimd.partition_broadcast(rb[:], rcp[:])
                    nc.vector.tensor_tensor(
                        ctxT[
                            hh * DK : hh * DK + DK, hp,
                            qc * 512 : (qc + 1) * 512,
                        ],
                        pc[0:DK, :],
                        rb[:],
                        OP.mult,
                    )

            # =================== attention + overlapped C/FFN ===============
            with (
                tc.tile_pool(name="xtp", bufs=1) as xtp,
                tc.tile_pool(name="wp", bufs=1) as wp,
            ):
                xt = xtp.tile([P, NE, S], fp8)
                wv_sb = wp.tile([P, NE, D], fp8, tag="wv")
                for et in range(NE):
                    nc.sync.dma_start(
                        wv_sb[:, et, :], wv_d.ap()[et * P : (et + 1) * P, :]
                    )
                    nc.sync.dma_start(
                        xt[:, et, :], xt_d.ap()[et * P : (et + 1) * P, :]
                    )
                wq_sb = wp.tile([P, NE, D], fp8, tag="wq")
                wk_sb = wp.tile([P, NE, D], fp8, tag="wk")
                for et in range(NE):
                    nc.sync.dma_start(
                        wq_sb[:, et, :], wq_d.ap()[et * P : (et + 1) * P, :]
                    )
                    nc.sync.dma_start(
                        wk_sb[:, et, :], wk_d.ap()[et * P : (et + 1) * P, :]
                    )

                # ones columns of V' (1.0; with wv x32 and pt=p/16 the
                # normalize yields ctxT = 32*ctx)
                ones192 = xtp.tile([P, NS * H], f32, tag="ones192")
                nc.gpsimd.memset(ones192[:], 1.0)
                nc.vector.tensor_copy(
                    vo5[:, :, :, :, DK : DK + 1],
                    ones192[:].rearrange(
                        "p (s j h o) -> p s j h o", s=NSP, j=2, h=H
                    ),
                )

                def v_chunk(st):
                    for dc, cw in ((0, 512), (512, 256)):
                        ps = psX.tile([P, 512], f32, tag="x")
                        for ep in range(NEP):
                            nc.tensor.matmul(
                                ps[:, 0:cw],
                                xt[:, 2 * ep : 2 * ep + 2, st * P : (st + 1) * P],
                                wv_sb[:, 2 * ep : 2 * ep + 2, dc : dc + cw],
                                start=(ep == 0),
                                stop=(ep == NEP - 1),
                                perf_mode=DR,
                            )
                        h0, nh = dc // DK, cw // DK
                        nc.vector.tensor_copy(
                            vo5[:, st // 2, st % 2, h0 : h0 + nh, 0:DK],
                            ps[:, 0:cw].rearrange("p (h w) -> p h w", w=DK),
                        )

                def q_proj(hp):
                    qh = kqr.tile([P, QR], fp8, tag="qh")
                    qh_t[hp] = qh
                    for qc in range(QR // 512):
                        ps = psX.tile([P, 512], f32, tag="x")
                        for ep in range(NEP):
                            nc.tensor.matmul(
                                ps[:],
                                wq_sb[:, 2 * ep : 2 * ep + 2, hp * P : (hp + 1) * P],
                                xt[:, 2 * ep : 2 * ep + 2, qc * 512 : (qc + 1) * 512],
                                start=(ep == 0),
                                stop=(ep == NEP - 1),
                                perf_mode=DR,
                            )
                        nc.vector.tensor_copy(qh[:, qc * 512 : (qc + 1) * 512], ps[:])

                def k_proj(hp):
                    kh0 = kqr.tile([P, S], fp8, tag="kh0")
                    kh1 = kqr.tile([P, S], fp8, tag="kh1")
                    kh_t[hp] = (kh0, kh1)
                    nc.gpsimd.memset(kh0[DK:P, :], 0.0)
                    nc.gpsimd.memset(kh1[0:DK, :], 0.0)
                    for sc in range(S // 512):
                        ps = psX.tile([P, 512], f32, tag="x")
                        for ep in range(NEP):
                            nc.tensor.matmul(
                                ps[:],
                                wk_sb[:, 2 * ep : 2 * ep + 2, hp * P : (hp + 1) * P],
                                xt[:, 2 * ep : 2 * ep + 2, sc * 512 : (sc + 1) * 512],
                                start=(ep == 0),
                                stop=(ep == NEP - 1),
                                perf_mode=DR,
                            )
                        nc.vector.tensor_copy(
                            kh0[0:DK, sc * 512 : (sc + 1) * 512], ps[0:DK, :]
                        )
                        nc.vector.tensor_copy(
                            kh1[DK:P, sc * 512 : (sc + 1) * 512], ps[DK:P, :]
                        )

                q_proj(0)
                k_proj(0)
                for et in range(NE):
                    nc.sync.dma_start(
                        wo_sb[:, et, :], wo_d.ap()[et * P : (et + 1) * P, :]
                    )

                # ------------- qc = 0 pass (V-proj + Q/K fillers) ----------
                for hp in range(H // 2):
                    pc0 = psC.tile([DK + 1, 512], f32, tag="c0")
                    pc1 = psC.tile([DK + 1, 512], f32, tag="c1")
                    pcs[0], pcs[1] = pc0, pc1
                    for ktp in range(NSP):
                        if hp == 0:
                            v_chunk(2 * ktp)
                            v_chunk(2 * ktp + 1)
                        kt_body(hp, 0, ktp)
                    if hp + 1 < H // 2:
                        q_proj(hp + 1)
                        k_proj(hp + 1)
                    chain(hp, 0)
            # xt / wq / wk / wv freed here

            # residual rows for the first half + w1 chunk prefetch
            xh_sb0 = xhp.tile([P, 4, D], bf16, tag="xh")
            for qt_i in range(4):
                nc.sync.dma_start(
                    xh_sb0[:, qt_i, :], xh_d.ap()[qt_i * P : (qt_i + 1) * P, :]
                )
            w1_dma(0, 0)
            w1_dma(0, 1)

            # work items hidden inside the qc=1 attention pass
            items = (
                [("c", qt) for qt in range(4)]
                + [("n", 0), ("n", 0)]
                + [("w1", f) for f in range(NF)]
            )
            it = [0]

            def emit_item():
                if it[0] >= len(items):
                    return
                kind, a = items[it[0]]
                it[0] += 1
                if kind == "c":
                    c_item(a, xh_sb0, 0)
                elif kind == "w1":
                    w1_item(0, a)

            # ------------------------- qc = 1 pass ---------------------------
            for hp in range(H // 2):
                pc0 = psC.tile([DK + 1, 512], f32, tag="c0")
                pc1 = psC.tile([DK + 1, 512], f32, tag="c1")
                pcs[0], pcs[1] = pc0, pc1
                for ktp in range(NSP):
                    kt_body(hp, 1, ktp)
                    emit_item()
                chain(hp, 1)
            while it[0] < len(items):
                emit_item()
            es_ab.close()  # free vo / qh / kh / pt / scores+ctx psum

            # ------------------------------ tail -----------------------------
            with (
                tc.tile_pool(name="w2p", bufs=1) as w2p,
                tc.tile_pool(name="sp2", bufs=3) as sp2,
                tc.tile_pool(name="psF", bufs=4, space="PSUM") as psF,
            ):
                w2_sb = w2p.tile([P, NF, D], bf16)
                for fc in range(NF // 4):
                    nc.sync.dma_start(
                        w2_sb[:, fc * 4 : (fc + 1) * 4, :],
                        dram3(w2_d.ap()[fc * 4 * P : (fc + 1) * 4 * P, :]),
                    )
                xh_sb1 = xhp.tile([P, 4, D], bf16, tag="xh")
                for qt_i in range(4):
                    nc.sync.dma_start(
                        xh_sb1[:, qt_i, :],
                        xh_d.ap()[(qt_i + 4) * P : (qt_i + 5) * P, :],
                    )
                w1_dma(1, 0)
                w1_dma(1, 1)
                for qt_i in range(4, NQ):
                    c_item(qt_i, xh_sb1, 4)
                for f_t in range(NF):
                    w1_item(1, f_t)

                def w2_block(qt_i):
                    qc = qt_i // 4
                    ht = ht_t[qc]
                    t2 = sp2.tile([P, D], f32, tag="t2")
                    for dc, cw in ((0, 512), (512, 256)):
                        ps = psF.tile([P, 512], f32, tag="psF")
                        for f_t in range(NF):
                            nc.tensor.matmul(
                                ps[:, 0:cw],
                                ht[:, f_t, (qt_i % 4) * P : (qt_i % 4 + 1) * P],
                                w2_sb[:, f_t, dc : dc + cw],
                                start=(f_t == 0),
                                stop=False,
                            )
                        nc.tensor.matmul(
                            ps[:, 0:cw],
                            ones_bf[0:1, 0:P],
                            b2_sb[0:1, dc : dc + cw],
                            start=False,
                            stop=True,
                        )
                        nc.vector.tensor_add(
                            t2[:, dc : dc + cw], x1[:, qt_i, dc : dc + cw],
                            ps[:, 0:cw],
                        )
                    osb = sp2.tile([P, D], f32, tag="osb")
                    layer_norm(t2[:], osb[:], l2a, l2b, sp2)
                    nc.sync.dma_start(
                        out_d.ap()[qt_i * P : (qt_i + 1) * P, :], osb[:]
                    )

                for qt_i in range(4):
                    w2_block(qt_i)
                for qt_i in range(4, NQ):
                    w2_block(qt_i)

    nc.compile()
    return nc


def _prep_in_maps(inputs):
    import ml_dtypes

    fp8 = ml_dtypes.float8_e4m3

    x = np.asarray(inputs["x"], dtype=np.float32)
    wq = np.ascontiguousarray(
        (np.asarray(inputs["wq"], np.float32) * 32.0).astype(fp8)
    )
    wk = np.ascontiguousarray(
        (np.asarray(inputs["wk"], np.float32) * 32.0).astype(fp8)
    )
    wv = np.ascontiguousarray(
        (np.asarray(inputs["wv"], np.float32) * 32.0).astype(fp8)
    )
    wo = np.ascontiguousarray(
        (np.asarray(inputs["wo"], np.float32) * 64.0).astype(fp8)
    )
    w1 = np.ascontiguousarray(
        np.asarray(inputs["w1"], np.float32).astype(ml_dtypes.bfloat16)
    )
    w2 = np.ascontiguousarray(
        np.asarray(inputs["w2"], np.float32).astype(ml_dtypes.bfloat16)
    )
    b1t = np.ascontiguousarray(
        np.asarray(inputs["b1"], np.float32).reshape(NF, P).T
    )
    b2r = np.ascontiguousarray(
        np.asarray(inputs["b2"], np.float32).reshape(1, D).astype(ml_dtypes.bfloat16)
    )
    ln1a = np.ascontiguousarray(
        np.broadcast_to(np.asarray(inputs["ln1_alpha"], np.float32), (P, D))
    )
    ln1b = np.ascontiguousarray(
        np.broadcast_to(np.asarray(inputs["ln1_bias"], np.float32), (P, D))
    )
    ln2a = np.ascontiguousarray(
        np.broadcast_to(np.asarray(inputs["ln2_alpha"], np.float32), (P, D))
    )
    ln2b = np.ascontiguousarray(
        np.broadcast_to(np.asarray(inputs["ln2_bias"], np.float32), (P, D))
    )
    shared = dict(
        wq=wq, wk=wk, wv=wv, wo=wo, w1=w1, w2=w2,
        b1t=b1t, b2r=b2r, ln1a=ln1a, ln1b=ln1b, ln2a=ln2a, ln2b=ln2b,
    )
    in_maps = []
    for c in range(NCORES):
        b, half = c // 2, c % 2
        xb = x[b]  # [S, D]
        rolled = np.concatenate([xb[half * QR :], xb[: half * QR]], axis=0)
        m = dict(shared)
        m["xt"] = np.ascontiguousarray(rolled.T.astype(fp8))
        m["xh"] = np.ascontiguousarray(
            xb[half * QR : half * QR + QR].astype(ml_dtypes.bfloat16)
        )
        in_maps.append(m)
    return in_maps


def _skip_affine(inputs):
    return (
        np.all(np.asarray(inputs["ln1_alpha"]) == 1.0)
        and np.all(np.asarray(inputs["ln2_alpha"]) == 1.0)
        and np.all(np.asarray(inputs["ln1_bias"]) == 0.0)
        and np.all(np.asarray(inputs["ln2_bias"]) == 0.0)
    )


def kernel(**inputs):
    from concourse.bass_utils import run_bass_kernel_spmd

    sa = bool(_skip_affine(inputs))
    key = ("nc", sa)
    if key not in _CACHE:
        _CACHE[key] = _build(sa)
    nc = _CACHE[key]
    in_maps = _prep_in_maps(inputs)
    res = run_bass_kernel_spmd(nc, in_maps, core_ids=list(range(NCORES)))
    out = np.empty((B, S, D), dtype=np.float32)
    for c in range(NCORES):
        b, half = c // 2, c % 2
        out[b, half * QR : half * QR + QR, :] = res.results[c]["out"]
    return out


# revision 38
# speedup vs baseline: 1.0358x; 1.0358x over previous
"""Trainium2 Bass kernel for a dense transformer encoder block.

Sharding: pure data-parallel, zero collectives. 8 cores; core c handles
batch b = c//2, query rows half = c%2 (1024 of 2048 seq positions).
Each core receives the full (sequence-rotated) x[b]^T so it can compute
K/V over all 2048 keys locally; queries are always columns 0:1024 of the
rotated x^T (attention is permutation-invariant over the key axis).

v3: query-chunk-outer attention with software-pipelined overlap: during
the second query-chunk's attention pass (which is paced by the Scalar
engine's softmax exps), the PE queue is fed the first chunk's wo
projection + LN1 + transposes and its full w1 FFN layer, one work item
per kt-pair slot.  fp8e4 DoubleRow matmuls (2 k-tiles per instruction)
for the V/Q/K projections and the attention*V matmul; fp8 scores
operands and fp8 ctxT/wo.  All fp8 prescales are powers of two and are
compensated exactly:
  - wq,wk x32 -> qh/kh store 32q/32k in fp8; scores psum = 1024*(q.k);
    exp runs with scale=2^-13 and bias=-ln16 (pt = p/16, max ~42 << 240).
  - wv x32, ones-column of V' = 1.0 -> ctx psum rows = 2*sum(p)v, denom
    row = sum(p)/16; reciprocal-normalize yields ctxT = 32*ctx in fp8.
  - wo x64 in fp8 -> wo psum = 2048*attn_out; one fused DVE op computes
    psum*2^-11 + x residual.
Scores run K=128 against zero-padded kh tiles (no PE tiling-mode
switches, keeps ctx DoubleRow weight-loads hidden).  w1 streams in
double-buffered chunks; w2 is resident only for the tail.
"""

import sys

if "/opt/trn_rl_repo" not in sys.path:
    sys.path.insert(0, "/opt/trn_rl_repo")

import numpy as np

B, S, D, H, DK, DFF = 4, 2048, 768, 12, 64, 3072
NCORES = 8
QR = 1024  # query rows per core
EPS = 1e-6
P = 128
NE = D // P  # 6 e-tiles (contraction over model dim)
NEP = NE // 2  # 3 DoubleRow pairs
NS = S // P  # 16 s-tiles (key positions)
NSP = NS // 2  # 8 DoubleRow kt pairs
NQ = QR // P  # 8 q-tiles
NF = DFF // P  # 24 f-tiles
HW = 80  # padded per-head width in V' (64 d + ones col + pad)
LN16 = 2.772588722239781  # ln(16)

_CACHE = {}


def _build(skip_affine):
    from contextlib import ExitStack

    import concourse.bass as bass
    import concourse.tile as tile
    from concourse import bacc, mybir
    from concourse.masks import make_identity

    dt = mybir.dt
    f32 = dt.float32
    bf16 = dt.bfloat16
    fp8 = dt.float8e4
    AF = mybir.ActivationFunctionType
    OP = mybir.AluOpType
    DR = mybir.MatmulPerfMode.DoubleRow

    nc = bacc.Bacc("TRN2", target_bir_lowering=False, debug=False)

    xt_d = nc.dram_tensor("xt", [D, S], fp8, kind="ExternalInput")
    xh_d = nc.dram_tensor("xh", [QR, D], bf16, kind="ExternalInput")
    wq_d = nc.dram_tensor("wq", [D, D], fp8, kind="ExternalInput")  # x32
    wk_d = nc.dram_tensor("wk", [D, D], fp8, kind="ExternalInput")  # x32
    wv_d = nc.dram_tensor("wv", [D, D], fp8, kind="ExternalInput")  # x32
    wo_d = nc.dram_tensor("wo", [D, D], fp8, kind="ExternalInput")  # x64
    w1_d = nc.dram_tensor("w1", [D, DFF], bf16, kind="ExternalInput")
    w2_d = nc.dram_tensor("w2", [DFF, D], bf16, kind="ExternalInput")
    b1_d = nc.dram_tensor("b1t", [P, NF], f32, kind="ExternalInput")  # b1 T'd
    b2_d = nc.dram_tensor("b2r", [1, D], bf16, kind="ExternalInput")
    ln1a_d = nc.dram_tensor("ln1a", [P, D], f32, kind="ExternalInput")  # bcast
    ln1b_d = nc.dram_tensor("ln1b", [P, D], f32, kind="ExternalInput")
    ln2a_d = nc.dram_tensor("ln2a", [P, D], f32, kind="ExternalInput")
    ln2b_d = nc.dram_tensor("ln2b", [P, D], f32, kind="ExternalInput")
    out_d = nc.dram_tensor("out", [QR, D], f32, kind="ExternalOutput")

    def dram3(d_ap, p=P):
        return d_ap.rearrange("(n p) s -> p n s", p=p)

    with tile.TileContext(nc) as tc:
        with ExitStack() as ctx:
            const = ctx.enter_context(tc.tile_pool(name="const", bufs=1))
            ones_bf = const.tile([1, P], bf16)
            nc.gpsimd.memset(ones_bf[:], 1.0)
            ident = const.tile([P, P], f32)
            make_identity(nc, ident[:])
            b1_sb = const.tile([P, NF], f32)
            nc.sync.dma_start(b1_sb[:], b1_d.ap())
            b2_sb = const.tile([1, D], bf16)
            nc.sync.dma_start(b2_sb[:], b2_d.ap())
            expb = const.tile([P, 1], f32)  # exp bias: -ln(16)
            nc.gpsimd.memset(expb[:], -LN16)

            # persistent (whole kernel) left-side pools
            xp = ctx.enter_context(tc.tile_pool(name="xp", bufs=1))
            x1 = xp.tile([P, NQ, D], f32, tag="x1")
            x1t = xp.tile([P, NE, QR], bf16, tag="x1t")
            xhp = ctx.enter_context(tc.tile_pool(name="xhp", bufs=1))
            spL = ctx.enter_context(tc.tile_pool(name="spL", bufs=1))
            lnc = ctx.enter_context(tc.tile_pool(name="lnc", bufs=1))
            if skip_affine:
                l1a = l1b = l2a = l2b = None
            else:
                l1a = lnc.tile([P, D], f32, tag="l1a")
                l1b = lnc.tile([P, D], f32, tag="l1b")
                l2a = lnc.tile([P, D], f32, tag="l2a")
                l2b = lnc.tile([P, D], f32, tag="l2b")
                nc.sync.dma_start(l1a[:], ln1a_d.ap())
                nc.sync.dma_start(l1b[:], ln1b_d.ap())
                nc.sync.dma_start(l2a[:], ln2a_d.ap())
                nc.sync.dma_start(l2b[:], ln2b_d.ap())
            w1p = ctx.enter_context(tc.tile_pool(name="w1p", bufs=2))
            # bufs=2: ht(qc1) relu must not wait on ht(qc0)'s w2 readers,
            # which sit later in the PE queue (would deadlock at bufs=1)
            htp = ctx.enter_context(tc.tile_pool(name="htp", bufs=2))

            # right side: ctx^T (fp8, = 32*ctx) + wo (fp8, = 64*wo)
            ctp = ctx.enter_context(tc.tile_pool(name="ctp", bufs=1, side="right"))
            ctxT = ctp.tile([P, NE, QR], fp8, tag="ctxT")
            wo_sb = ctp.tile([P, NE, D], fp8, tag="wo")

            psX = ctx.enter_context(tc.tile_pool(name="psX", bufs=2, space="PSUM"))

            es_ab = ExitStack()  # attention-lifetime pools
            kqv = es_ab.enter_context(tc.tile_pool(name="kqv", bufs=1))
            vo = kqv.tile([P, NSP, 2, H * HW], fp8, tag="vo")
            vo5 = vo[:, :, :, :].rearrange("p s j (h w) -> p s j h w", w=HW)
            kqr = es_ab.enter_context(tc.tile_pool(name="kqr", bufs=6))
            ptp = es_ab.enter_context(tc.tile_pool(name="ptp", bufs=2))
            up = es_ab.enter_context(tc.tile_pool(name="up", bufs=2))
            psS = es_ab.enter_context(tc.tile_pool(name="psS", bufs=2, space="PSUM"))
            psC = es_ab.enter_context(tc.tile_pool(name="psC", bufs=1, space="PSUM"))

            def layer_norm(tin, out_ap, a_bc, b_bc, spool):
                st6 = spool.tile([P, 2, 6], f32, tag="st6")
                nc.vector.bn_stats(st6[:, 0, :], tin[:, 0:384])
                nc.vector.bn_stats(st6[:, 1, :], tin[:, 384:768])
                mv = spool.tile([P, 2], f32, tag="mv")
                nc.vector.bn_aggr(mv[:], st6[:])
                std = spool.tile([P, 1], f32, tag="std")
                nc.scalar.activation(
                    std[:], mv[:, 1:2], AF.Sqrt, scale=float(D) / (D - 1)
                )
                stde = spool.tile([P, 1], f32, tag="stde")
                nc.vector.tensor_scalar_add(stde[:], std[:], EPS)
                rstd = spool.tile([P, 1], f32, tag="rstd")
                nc.vector.reciprocal(rstd[:], stde[:])
                if skip_affine:
                    nc.vector.tensor_scalar(
                        out_ap, tin[:], mv[:, 0:1], rstd[:],
                        op0=OP.subtract, op1=OP.mult,
                    )
                else:
                    yc = spool.tile([P, D], f32, tag="yc")
                    nc.vector.tensor_scalar(
                        yc[:], tin[:], mv[:, 0:1], rstd[:],
                        op0=OP.subtract, op1=OP.mult,
                    )
                    y2 = spool.tile([P, D], f32, tag="y2")
                    nc.vector.tensor_tensor(y2[:], yc[:], a_bc, OP.mult)
                    nc.vector.tensor_tensor(out_ap, y2[:], b_bc, OP.add)

            w1tiles = {}

            def w1_dma(qc, c):
                t = w1p.tile([P, NE, 4 * P], bf16, tag="w1c")
                for et in range(NE):
                    nc.sync.dma_start(
                        t[:, et, :],
                        w1_d.ap()[
                            et * P : (et + 1) * P, c * 4 * P : (c + 1) * 4 * P
                        ],
                    )
                w1tiles[(qc, c)] = t

            ht_t = [None, None]

            def w1_item(qc, f_t):
                c = f_t // 4
                if f_t % 4 == 0 and c + 2 < NF // 4:
                    w1_dma(qc, c + 2)
                if f_t == 0:
                    ht_new = htp.tile([P, NF, 512], bf16, tag="ht")
                    ht_t[qc] = ht_new
                ht = ht_t[qc]
                w1c = w1tiles[(qc, c)]
                ps = psX.tile([P, 512], f32, tag="x")
                for et in range(NE):
                    nc.tensor.matmul(
                        ps[:],
                        w1c[:, et, (f_t % 4) * P : (f_t % 4 + 1) * P],
                        x1t[:, et, qc * 512 : (qc + 1) * 512],
                        start=(et == 0),
                        stop=(et == NE - 1),
                    )
                nc.scalar.activation(
                    ht[:, f_t, :], ps[:], AF.Relu, bias=b1_sb[:, f_t : f_t + 1]
                )

            def c_item_a(qt_i, xh_sb, xo):
                # wo projection (fp8: psum = 2048*attn) + residual + LN1
                tsb = spL.tile([P, D], f32, tag="tsb")
                for dc, cw in ((0, 512), (512, 256)):
                    ps = psX.tile([P, 512], f32, tag="x")
                    for dt_i in range(NE):
                        nc.tensor.matmul(
                            ps[:, 0:cw],
                            ctxT[:, dt_i, qt_i * P : (qt_i + 1) * P],
                            wo_sb[:, dt_i, dc : dc + cw],
                            start=(dt_i == 0),
                            stop=(dt_i == NE - 1),
                        )
                    nc.vector.scalar_tensor_tensor(
                        tsb[:, dc : dc + cw],
                        ps[:, 0:cw],
                        float(2.0 ** -11),
                        xh_sb[:, qt_i - xo, dc : dc + cw],
                        op0=OP.mult,
                        op1=OP.add,
                    )
                layer_norm(tsb[:], x1[:, qt_i, :], l1a, l1b, spL)

            def c_item_b(qt_i):
                # x1^T transposes, emitted a couple of slots after c_item_a
                # so the PE never waits on the LN chain
                for dt_i in range(NE):
                    pst = psX.tile([P, 512], f32, tag="x")
                    nc.tensor.transpose(
                        pst[:, 0:P], x1[:, qt_i, dt_i * P : (dt_i + 1) * P],
                        ident[:],
                    )
                    nc.vector.tensor_copy(
                        x1t[:, dt_i, qt_i * P : (qt_i + 1) * P], pst[:, 0:P]
                    )

            def c_item(qt_i, xh_sb, xo):
                c_item_a(qt_i, xh_sb, xo)
                c_item_b(qt_i)

            qh_t = [None] * (H // 2)
            kh_t = [None] * (H // 2)
            pcs = [None, None]

            def kt_body(hp, qc, ktp, fill=None):
                qh = qh_t[hp]
                kh0, kh1 = kh_t[hp]
                pc0, pc1 = pcs
                pt = ptp.tile([P, 2, 1024], fp8, tag="pt")
                for j in range(2):
                    kt_i = 2 * ktp + j
                    ps = psS.tile([P, 1024], f32, tag="psS")
                    for hh, khp in ((0, kh0), (1, kh1)):
                        nc.tensor.matmul(
                            ps[:, hh * 512 : hh * 512 + 512],
                            khp[:, kt_i * P : (kt_i + 1) * P],
                            qh[:, qc * 512 : (qc + 1) * 512],
                            start=True,
                            stop=True,
                        )
                    # pt = exp(scores)/16; scores psum = 1024*(q.k)
                    nc.scalar.activation(
                        pt[:, j, :], ps[:], AF.Exp,
                        bias=expb[:], scale=float(2.0 ** -13),
                    )
                if fill is not None:
                    # overlap work goes between the scores and the ctx
                    # matmuls: the PE would otherwise idle here waiting on
                    # the exps that the ctx matmuls consume
                    fill()
                for hh, pc in ((0, pc0), (1, pc1)):
                    h = 2 * hp + hh
                    nc.tensor.matmul(
                        pc[:],
                        vo5[:, ktp, :, h, 0 : DK + 1],
                        pt[:, :, hh * 512 : hh * 512 + 512],
                        start=(ktp == 0),
                        stop=(ktp == NSP - 1),
                        perf_mode=DR,
                    )

            def chain(hp, qc):
                # softmax normalize: ctxT = 32*ctx in fp8
                pc0, pc1 = pcs
                for hh, pc in ((0, pc0), (1, pc1)):
                    dcp = up.tile([1, 512], f32, tag="dcp")
                    nc.vector.tensor_copy(dcp[:], pc[DK : DK + 1, :])
                    rcp = up.tile([1, 512], f32, tag="rcp")
                    nc.vector.reciprocal_approx_fast(rcp[:], dcp[:])
                    rb = up.tile([DK, 512], f32, tag="rb")
                    nc.gpsimd.partition_broadcast(rb[:], rcp[:])
                    nc.vector.tensor_tensor(
                        ctxT[
                            hh * DK : hh * DK + DK, hp,
                            qc * 512 : (qc + 1) * 512,
                        ],
                        pc[0:DK, :],
                        rb[:],
                        OP.mult,
                    )

            # =================== attention + overlapped C/FFN ===============
            with (
                tc.tile_pool(name="xtp", bufs=1) as xtp,
                tc.tile_pool(name="wp", bufs=1) as wp,
            ):
                xt = xtp.tile([P, NE, S], fp8)
                wv_sb = wp.tile([P, NE, D], fp8, tag="wv")
                for et in range(NE):
                    nc.sync.dma_start(
                        wv_sb[:, et, :], wv_d.ap()[et * P : (et + 1) * P, :]
                    )
                    nc.sync.dma_start(
                        xt[:, et, :], xt_d.ap()[et * P : (et + 1) * P, :]
                    )
                wq_sb = wp.tile([P, NE, D], fp8, tag="wq")
                wk_sb = wp.tile([P, NE, D], fp8, tag="wk")
                for et in range(NE):
                    nc.sync.dma_start(
                        wq_sb[:, et, :], wq_d.ap()[et * P : (et + 1) * P, :]
                    )
                    nc.sync.dma_start(
                        wk_sb[:, et, :], wk_d.ap()[et * P : (et + 1) * P, :]
                    )

                # ones columns of V' (1.0; with wv x32 and pt=p/16 the
                # normalize yields ctxT = 32*ctx)
                ones192 = xtp.tile([P, NS * H], f32, tag="ones192")
                nc.gpsimd.memset(ones192[:], 1.0)
                nc.vector.tensor_copy(
                    vo5[:, :, :, :, DK : DK + 1],
                    ones192[:].rearrange(
                        "p (s j h o) -> p s j h o", s=NSP, j=2, h=H
                    ),
                )

                def v_chunk(st):
                    for dc, cw in ((0, 512), (512, 256)):
                        ps = psX.tile([P, 512], f32, tag="x")
                        for ep in range(NEP):
                            nc.tensor.matmul(
                                ps[:, 0:cw],
                                xt[:, 2 * ep : 2 * ep + 2, st * P : (st + 1) * P],
                                wv_sb[:, 2 * ep : 2 * ep + 2, dc : dc + cw],
                                start=(ep == 0),
                                stop=(ep == NEP - 1),
                                perf_mode=DR,
                            )
                        h0, nh = dc // DK, cw // DK
                        nc.vector.tensor_copy(
                            vo5[:, st // 2, st % 2, h0 : h0 + nh, 0:DK],
                            ps[:, 0:cw].rearrange("p (h w) -> p h w", w=DK),
                        )

                def q_proj(hp):
                    qh = kqr.tile([P, QR], fp8, tag="qh")
                    qh_t[hp] = qh
                    for qc in range(QR // 512):
                        ps = psX.tile([P, 512], f32, tag="x")
                        for ep in range(NEP):
                            nc.tensor.matmul(
                                ps[:],
                                wq_sb[:, 2 * ep : 2 * ep + 2, hp * P : (hp + 1) * P],
                                xt[:, 2 * ep : 2 * ep + 2, qc * 512 : (qc + 1) * 512],
                                start=(ep == 0),
                                stop=(ep == NEP - 1),
                                perf_mode=DR,
                            )
                        nc.vector.tensor_copy(qh[:, qc * 512 : (qc + 1) * 512], ps[:])

                def k_proj(hp):
                    kh0 = kqr.tile([P, S], fp8, tag="kh0")
                    kh1 = kqr.tile([P, S], fp8, tag="kh1")
                    kh_t[hp] = (kh0, kh1)
                    nc.gpsimd.memset(kh0[DK:P, :], 0.0)
                    nc.gpsimd.memset(kh1[0:DK, :], 0.0)
                    for sc in range(S // 512):
                        ps = psX.tile([P, 512], f32, tag="x")
                        for ep in range(NEP):
                            nc.tensor.matmul(
                                ps[:],
                                wk_sb[:, 2 * ep : 2 * ep + 2, hp * P : (hp + 1) * P],
                                xt[:, 2 * ep : 2 * ep + 2, sc * 512 : (sc + 1) * 512],
                                start=(ep == 0),
                                stop=(ep == NEP - 1),
                                perf_mode=DR,
                            )
                        nc.vector.tensor_copy(
                            kh0[0:DK, sc * 512 : (sc + 1) * 512], ps[0:DK, :]
                        )
                        nc.vector.tensor_copy(
                            kh1[DK:P, sc * 512 : (sc + 1) * 512], ps[DK:P, :]
                        )

                q_proj(0)
                k_proj(0)
                for et in range(NE):
                    nc.sync.dma_start(
                        wo_sb[:, et, :], wo_d.ap()[et * P : (et + 1) * P, :]
                    )

                # ------------- qc = 0 pass (V-proj + Q/K fillers) ----------
                for hp in range(H // 2):
                    pc0 = psC.tile([DK + 1, 512], f32, tag="c0")
                    pc1 = psC.tile([DK + 1, 512], f32, tag="c1")
                    pcs[0], pcs[1] = pc0, pc1
                    for ktp in range(NSP):
                        if hp == 0:
                            v_chunk(2 * ktp)
                            v_chunk(2 * ktp + 1)
                        kt_body(hp, 0, ktp)
                    if hp + 1 < H // 2:
                        q_proj(hp + 1)
                        k_proj(hp + 1)
                    chain(hp, 0)
            # xt / wq / wk / wv freed here

            # residual rows for the first half + w1 chunk prefetch
            xh_sb0 = xhp.tile([P, 4, D], bf16, tag="xh")
            for qt_i in range(4):
                nc.sync.dma_start(
                    xh_sb0[:, qt_i, :], xh_d.ap()[qt_i * P : (qt_i + 1) * P, :]
                )
            w1_dma(0, 0)
            w1_dma(0, 1)

            # work items hidden inside the qc=1 attention pass; transposes
            # trail their LN by two slots to hide the DVE chain latency
            items = (
                [("ca", 0), ("ca", 1), ("cb", 0), ("ca", 2), ("cb", 1),
                 ("ca", 3), ("cb", 2), ("cb", 3)]
                + [("w1", f) for f in range(NF)]
            )
            it = [0]

            def emit_item():
                if it[0] >= len(items):
                    return
                kind, a = items[it[0]]
                it[0] += 1
                if kind == "ca":
                    c_item_a(a, xh_sb0, 0)
                elif kind == "cb":
                    c_item_b(a)
                elif kind == "w1":
                    w1_item(0, a)

            # ------------------------- qc = 1 pass ---------------------------
            for hp in range(H // 2):
                pc0 = psC.tile([DK + 1, 512], f32, tag="c0")
                pc1 = psC.tile([DK + 1, 512], f32, tag="c1")
                pcs[0], pcs[1] = pc0, pc1
                for ktp in range(NSP):
                    kt_body(hp, 1, ktp, fill=emit_item)
                chain(hp, 1)
            while it[0] < len(items):
                emit_item()
            es_ab.close()  # free vo / qh / kh / pt / scores+ctx psum

            # ------------------------------ tail -----------------------------
            with (
                tc.tile_pool(name="w2p", bufs=1) as w2p,
                tc.tile_pool(name="sp2", bufs=3) as sp2,
                tc.tile_pool(name="psF", bufs=4, space="PSUM") as psF,
            ):
                w2_sb = w2p.tile([P, NF, D], bf16)
                for fc in range(NF // 4):
                    nc.sync.dma_start(
                        w2_sb[:, fc * 4 : (fc + 1) * 4, :],
                        dram3(w2_d.ap()[fc * 4 * P : (fc + 1) * 4 * P, :]),
                    )
                xh_sb1 = xhp.tile([P, 4, D], bf16, tag="xh")
                for qt_i in range(4):
                    nc.sync.dma_start(
                        xh_sb1[:, qt_i, :],
                        xh_d.ap()[(qt_i + 4) * P : (qt_i + 5) * P, :],
                    )
                w1_dma(1, 0)
                w1_dma(1, 1)
                for qt_i in range(4, NQ):
                    c_item(qt_i, xh_sb1, 4)
                for f_t in range(NF):
                    w1_item(1, f_t)

                def w2_block(qt_i):
                    qc = qt_i // 4
                    ht = ht_t[qc]
                    t2 = sp2.tile([P, D], f32, tag="t2")
                    for dc, cw in ((0, 512), (512, 256)):
                        ps = psF.tile([P, 512], f32, tag="psF")
                        for f_t in range(NF):
                            nc.tensor.matmul(
                                ps[:, 0:cw],
                                ht[:, f_t, (qt_i % 4) * P : (qt_i % 4 + 1) * P],
                                w2_sb[:, f_t, dc : dc + cw],
                                start=(f_t == 0),
                                stop=False,
                            )
                        nc.tensor.matmul(
                            ps[:, 0:cw],
                            ones_bf[0:1, 0:P],
                            b2_sb[0:1, dc : dc + cw],
                            start=False,
                            stop=True,
                        )
                        nc.vector.tensor_add(
                            t2[:, dc : dc + cw], x1[:, qt_i, dc : dc + cw],
                            ps[:, 0:cw],
                        )
                    osb = sp2.tile([P, D], f32, tag="osb")
                    layer_norm(t2[:], osb[:], l2a, l2b, sp2)
                    nc.sync.dma_start(
                        out_d.ap()[qt_i * P : (qt_i + 1) * P, :], osb[:]
                    )

                for qt_i in range(4):
                    w2_block(qt_i)
                for qt_i in range(4, NQ):
                    w2_block(qt_i)

    nc.compile()
    return nc


def _prep_in_maps(inputs):
    import ml_dtypes

    fp8 = ml_dtypes.float8_e4m3

    x = np.asarray(inputs["x"], dtype=np.float32)
    wq = np.ascontiguousarray(
        (np.asarray(inputs["wq"], np.float32) * 32.0).astype(fp8)
    )
    wk = np.ascontiguousarray(
        (np.asarray(inputs["wk"], np.float32) * 32.0).astype(fp8)
    )
    wv = np.ascontiguousarray(
        (np.asarray(inputs["wv"], np.float32) * 32.0).astype(fp8)
    )
    wo = np.ascontiguousarray(
        (np.asarray(inputs["wo"], np.float32) * 64.0).astype(fp8)
    )
    w1 = np.ascontiguousarray(
        np.asarray(inputs["w1"], np.float32).astype(ml_dtypes.bfloat16)
    )
    w2 = np.ascontiguousarray(
        np.asarray(inputs["w2"], np.float32).astype(ml_dtypes.bfloat16)
    )
    b1t = np.ascontiguousarray(
        np.asarray(inputs["b1"], np.float32).reshape(NF, P).T
    )
    b2r = np.ascontiguousarray(
        np.asarray(inputs["b2"], np.float32).reshape(1, D).astype(ml_dtypes.bfloat16)
    )
    ln1a = np.ascontiguousarray(
        np.broadcast_to(np.asarray(inputs["ln1_alpha"], np.float32), (P, D))
    )
    ln1b = np.ascontiguousarray(
        np.broadcast_to(np.asarray(inputs["ln1_bias"], np.float32), (P, D))
    )
    ln2a = np.ascontiguousarray(
        np.broadcast_to(np.asarray(inputs["ln2_alpha"], np.float32), (P, D))
    )
    ln2b = np.ascontiguousarray(
        np.broadcast_to(np.asarray(inputs["ln2_bias"], np.float32), (P, D))
    )
    shared = dict(
        wq=wq, wk=wk, wv=wv, wo=wo, w1=w1, w2=w2,
        b1t=b1t, b2r=b2r, ln1a=ln1a, ln1b=ln1b, ln2a=ln2a, ln2b=ln2b,
    )
    in_maps = []
    for c in range(NCORES):
        b, half = c // 2, c % 2
        xb = x[b]  # [S, D]
        rolled = np.concatenate([xb[half * QR :], xb[: half * QR]], axis=0)
        m = dict(shared)
        m["xt"] = np.ascontiguousarray(rolled.T.astype(fp8))
        m["xh"] = np.ascontiguousarray(
            xb[half * QR : half * QR + QR].astype(ml_dtypes.bfloat16)
        )
        in_maps.append(m)
    return in_maps


def _skip_affine(inputs):
    return (
        np.all(np.asarray(inputs["ln1_alpha"]) == 1.0)
        and np.all(np.asarray(inputs["ln2_alpha"]) == 1.0)
        and np.all(np.asarray(inputs["ln1_bias"]) == 0.0)
        and np.all(np.asarray(inputs["ln2_bias"]) == 0.0)
    )


def kernel(**inputs):
    from concourse.bass_utils import run_bass_kernel_spmd

    sa = bool(_skip_affine(inputs))
    key = ("nc", sa)
    if key not in _CACHE:
        _CACHE[key] = _build(sa)
    nc = _CACHE[key]
    in_maps = _prep_in_maps(inputs)
    res = run_bass_kernel_spmd(nc, in_maps, core_ids=list(range(NCORES)))
    out = np.empty((B, S, D), dtype=np.float32)
    for c in range(NCORES):
        b, half = c // 2, c % 2
        out[b, half * QR : half * QR + QR, :] = res.results[c]["out"]
    return out


# revision 41
# speedup vs baseline: 1.0435x; 1.0074x over previous
"""Trainium2 Bass kernel for a dense transformer encoder block.

Sharding: pure data-parallel, zero collectives. 8 cores; core c handles
batch b = c//2, query rows half = c%2 (1024 of 2048 seq positions).
Each core receives the full (sequence-rotated) x[b]^T so it can compute
K/V over all 2048 keys locally; queries are always columns 0:1024 of the
rotated x^T (attention is permutation-invariant over the key axis).

v3: query-chunk-outer attention with software-pipelined overlap: during
the second query-chunk's attention pass (which is paced by the Scalar
engine's softmax exps), the PE queue is fed the first chunk's wo
projection + LN1 + transposes and its full w1 FFN layer, one work item
per kt-pair slot.  fp8e4 DoubleRow matmuls (2 k-tiles per instruction)
for the V/Q/K projections and the attention*V matmul; fp8 scores
operands and fp8 ctxT/wo.  All fp8 prescales are powers of two and are
compensated exactly:
  - wq,wk x32 -> qh/kh store 32q/32k in fp8; scores psum = 1024*(q.k);
    exp runs with scale=2^-13 and bias=-ln16 (pt = p/16, max ~42 << 240).
  - wv x32, ones-column of V' = 1.0 -> ctx psum rows = 2*sum(p)v, denom
    row = sum(p)/16; reciprocal-normalize yields ctxT = 32*ctx in fp8.
  - wo x64 in fp8 -> wo psum = 2048*attn_out; one fused DVE op computes
    psum*2^-11 + x residual.
Scores run K=128 against zero-padded kh tiles (no PE tiling-mode
switches, keeps ctx DoubleRow weight-loads hidden).  w1 streams in
double-buffered chunks; w2 is resident only for the tail.
"""

import sys

if "/opt/trn_rl_repo" not in sys.path:
    sys.path.insert(0, "/opt/trn_rl_repo")

import numpy as np

B, S, D, H, DK, DFF = 4, 2048, 768, 12, 64, 3072
NCORES = 8
QR = 1024  # query rows per core
EPS = 1e-6
P = 128
NE = D // P  # 6 e-tiles (contraction over model dim)
NEP = NE // 2  # 3 DoubleRow pairs
NS = S // P  # 16 s-tiles (key positions)
NSP = NS // 2  # 8 DoubleRow kt pairs
NQ = QR // P  # 8 q-tiles
NF = DFF // P  # 24 f-tiles
HW = 80  # padded per-head width in V' (64 d + ones col + pad)
LN16 = 2.772588722239781  # ln(16)

_CACHE = {}


def _build(skip_affine):
    from contextlib import ExitStack

    import concourse.bass as bass
    import concourse.tile as tile
    from concourse import bacc, mybir
    from concourse.masks import make_identity

    dt = mybir.dt
    f32 = dt.float32
    bf16 = dt.bfloat16
    fp8 = dt.float8e4
    AF = mybir.ActivationFunctionType
    OP = mybir.AluOpType
    DR = mybir.MatmulPerfMode.DoubleRow

    nc = bacc.Bacc("TRN2", target_bir_lowering=False, debug=False)

    xt_d = nc.dram_tensor("xt", [D, S], fp8, kind="ExternalInput")
    xh_d = nc.dram_tensor("xh", [QR, D], bf16, kind="ExternalInput")
    wq_d = nc.dram_tensor("wq", [D, D], fp8, kind="ExternalInput")  # x32
    wk_d = nc.dram_tensor("wk", [D, D], fp8, kind="ExternalInput")  # x32
    wv_d = nc.dram_tensor("wv", [D, D], fp8, kind="ExternalInput")  # x32
    wo_d = nc.dram_tensor("wo", [D, D], fp8, kind="ExternalInput")  # x64
    w1_d = nc.dram_tensor("w1", [D, DFF], bf16, kind="ExternalInput")
    w2_d = nc.dram_tensor("w2", [DFF, D], bf16, kind="ExternalInput")
    b1_d = nc.dram_tensor("b1t", [P, NF], f32, kind="ExternalInput")  # b1 T'd
    b2_d = nc.dram_tensor("b2r", [1, D], bf16, kind="ExternalInput")
    ln1a_d = nc.dram_tensor("ln1a", [P, D], f32, kind="ExternalInput")  # bcast
    ln1b_d = nc.dram_tensor("ln1b", [P, D], f32, kind="ExternalInput")
    ln2a_d = nc.dram_tensor("ln2a", [P, D], f32, kind="ExternalInput")
    ln2b_d = nc.dram_tensor("ln2b", [P, D], f32, kind="ExternalInput")
    out_d = nc.dram_tensor("out", [QR, D], f32, kind="ExternalOutput")

    def dram3(d_ap, p=P):
        return d_ap.rearrange("(n p) s -> p n s", p=p)

    with tile.TileContext(nc) as tc:
        with ExitStack() as ctx:
            const = ctx.enter_context(tc.tile_pool(name="const", bufs=1))
            ones_bf = const.tile([1, P], bf16)
            nc.gpsimd.memset(ones_bf[:], 1.0)
            ident = const.tile([P, P], f32)
            make_identity(nc, ident[:])
            b1_sb = const.tile([P, NF], f32)
            nc.sync.dma_start(b1_sb[:], b1_d.ap())
            b2_sb = const.tile([1, D], bf16)
            nc.sync.dma_start(b2_sb[:], b2_d.ap())
            expb = const.tile([P, 1], f32)  # exp bias: -ln(16)
            nc.gpsimd.memset(expb[:], -LN16)

            # persistent (whole kernel) left-side pools
            xp = ctx.enter_context(tc.tile_pool(name="xp", bufs=1))
            x1 = xp.tile([P, NQ, D], f32, tag="x1")
            x1t = xp.tile([P, NE, QR], bf16, tag="x1t")
            xhp = ctx.enter_context(tc.tile_pool(name="xhp", bufs=1))
            spL = ctx.enter_context(tc.tile_pool(name="spL", bufs=1))
            lnc = ctx.enter_context(tc.tile_pool(name="lnc", bufs=1))
            if skip_affine:
                l1a = l1b = l2a = l2b = None
            else:
                l1a = lnc.tile([P, D], f32, tag="l1a")
                l1b = lnc.tile([P, D], f32, tag="l1b")
                l2a = lnc.tile([P, D], f32, tag="l2a")
                l2b = lnc.tile([P, D], f32, tag="l2b")
                nc.sync.dma_start(l1a[:], ln1a_d.ap())
                nc.sync.dma_start(l1b[:], ln1b_d.ap())
                nc.sync.dma_start(l2a[:], ln2a_d.ap())
                nc.sync.dma_start(l2b[:], ln2b_d.ap())
            w1p = ctx.enter_context(tc.tile_pool(name="w1p", bufs=2))
            # bufs=2: ht(qc1) relu must not wait on ht(qc0)'s w2 readers,
            # which sit later in the PE queue (would deadlock at bufs=1)
            htp = ctx.enter_context(tc.tile_pool(name="htp", bufs=2))

            # right side: ctx^T (fp8, = 32*ctx) + wo (fp8, = 64*wo)
            ctp = ctx.enter_context(tc.tile_pool(name="ctp", bufs=1, side="right"))
            ctxT = ctp.tile([P, NE, QR], fp8, tag="ctxT")
            wo_sb = ctp.tile([P, NE, D], fp8, tag="wo")

            psX = ctx.enter_context(tc.tile_pool(name="psX", bufs=2, space="PSUM"))

            es_ab = ExitStack()  # attention-lifetime pools
            kqv = es_ab.enter_context(tc.tile_pool(name="kqv", bufs=1))
            vo = kqv.tile([P, NSP, 2, H * HW], fp8, tag="vo")
            vo5 = vo[:, :, :, :].rearrange("p s j (h w) -> p s j h w", w=HW)
            kqr = es_ab.enter_context(tc.tile_pool(name="kqr", bufs=6))
            ptp = es_ab.enter_context(tc.tile_pool(name="ptp", bufs=2))
            up = es_ab.enter_context(tc.tile_pool(name="up", bufs=2))
            psS = es_ab.enter_context(tc.tile_pool(name="psS", bufs=2, space="PSUM"))
            psC = es_ab.enter_context(tc.tile_pool(name="psC", bufs=1, space="PSUM"))

            def layer_norm(tin, out_ap, a_bc, b_bc, spool):
                st6 = spool.tile([P, 2, 6], f32, tag="st6")
                nc.vector.bn_stats(st6[:, 0, :], tin[:, 0:384])
                nc.vector.bn_stats(st6[:, 1, :], tin[:, 384:768])
                mv = spool.tile([P, 2], f32, tag="mv")
                nc.vector.bn_aggr(mv[:], st6[:])
                std = spool.tile([P, 1], f32, tag="std")
                nc.scalar.activation(
                    std[:], mv[:, 1:2], AF.Sqrt, scale=float(D) / (D - 1)
                )
                stde = spool.tile([P, 1], f32, tag="stde")
                nc.vector.tensor_scalar_add(stde[:], std[:], EPS)
                rstd = spool.tile([P, 1], f32, tag="rstd")
                nc.vector.reciprocal(rstd[:], stde[:])
                if skip_affine:
                    nc.vector.tensor_scalar(
                        out_ap, tin[:], mv[:, 0:1], rstd[:],
                        op0=OP.subtract, op1=OP.mult,
                    )
                else:
                    yc = spool.tile([P, D], f32, tag="yc")
                    nc.vector.tensor_scalar(
                        yc[:], tin[:], mv[:, 0:1], rstd[:],
                        op0=OP.subtract, op1=OP.mult,
                    )
                    y2 = spool.tile([P, D], f32, tag="y2")
                    nc.vector.tensor_tensor(y2[:], yc[:], a_bc, OP.mult)
                    nc.vector.tensor_tensor(out_ap, y2[:], b_bc, OP.add)

            w1tiles = {}

            def w1_dma(qc, c):
                t = w1p.tile([P, NE, 4 * P], bf16, tag="w1c")
                for et in range(NE):
                    nc.sync.dma_start(
                        t[:, et, :],
                        w1_d.ap()[
                            et * P : (et + 1) * P, c * 4 * P : (c + 1) * 4 * P
                        ],
                    )
                w1tiles[(qc, c)] = t

            ht_t = [None, None]

            def w1_item(qc, f_t):
                c = f_t // 4
                if f_t % 4 == 0 and c + 2 < NF // 4:
                    w1_dma(qc, c + 2)
                if f_t == 0:
                    ht_new = htp.tile([P, NF, 512], bf16, tag="ht")
                    ht_t[qc] = ht_new
                ht = ht_t[qc]
                w1c = w1tiles[(qc, c)]
                ps = psX.tile([P, 512], f32, tag="x")
                for et in range(NE):
                    nc.tensor.matmul(
                        ps[:],
                        w1c[:, et, (f_t % 4) * P : (f_t % 4 + 1) * P],
                        x1t[:, et, qc * 512 : (qc + 1) * 512],
                        start=(et == 0),
                        stop=(et == NE - 1),
                    )
                nc.scalar.activation(
                    ht[:, f_t, :], ps[:], AF.Relu, bias=b1_sb[:, f_t : f_t + 1]
                )

            def c_item_a(qt_i, xh_sb, xo, spool=None):
                # wo projection (fp8: psum = 2048*attn) + residual + LN1
                spool = spool if spool is not None else spL
                tsb = spool.tile([P, D], f32, tag="tsb")
                for dc, cw in ((0, 512), (512, 256)):
                    ps = psX.tile([P, 512], f32, tag="x")
                    for dt_i in range(NE):
                        nc.tensor.matmul(
                            ps[:, 0:cw],
                            ctxT[:, dt_i, qt_i * P : (qt_i + 1) * P],
                            wo_sb[:, dt_i, dc : dc + cw],
                            start=(dt_i == 0),
                            stop=(dt_i == NE - 1),
                        )
                    nc.vector.scalar_tensor_tensor(
                        tsb[:, dc : dc + cw],
                        ps[:, 0:cw],
                        float(2.0 ** -11),
                        xh_sb[:, qt_i - xo, dc : dc + cw],
                        op0=OP.mult,
                        op1=OP.add,
                    )
                layer_norm(tsb[:], x1[:, qt_i, :], l1a, l1b, spool)

            def c_item_b(qt_i):
                # x1^T transposes, emitted a couple of slots after c_item_a
                # so the PE never waits on the LN chain
                for dt_i in range(NE):
                    pst = psX.tile([P, 512], f32, tag="x")
                    nc.tensor.transpose(
                        pst[:, 0:P], x1[:, qt_i, dt_i * P : (dt_i + 1) * P],
                        ident[:],
                    )
                    nc.vector.tensor_copy(
                        x1t[:, dt_i, qt_i * P : (qt_i + 1) * P], pst[:, 0:P]
                    )

            def c_item(qt_i, xh_sb, xo):
                c_item_a(qt_i, xh_sb, xo)
                c_item_b(qt_i)

            qh_t = [None] * (H // 2)
            kh_t = [None] * (H // 2)
            pcs = [None, None]

            def kt_body(hp, qc, ktp, fill=None):
                qh = qh_t[hp]
                kh0, kh1 = kh_t[hp]
                pc0, pc1 = pcs
                pt = ptp.tile([P, 2, 1024], fp8, tag="pt")
                for j in range(2):
                    kt_i = 2 * ktp + j
                    ps = psS.tile([P, 1024], f32, tag="psS")
                    for hh, khp in ((0, kh0), (1, kh1)):
                        nc.tensor.matmul(
                            ps[:, hh * 512 : hh * 512 + 512],
                            khp[:, kt_i * P : (kt_i + 1) * P],
                            qh[:, qc * 512 : (qc + 1) * 512],
                            start=True,
                            stop=True,
                        )
                    # pt = exp(scores)/16; scores psum = 1024*(q.k)
                    nc.scalar.activation(
                        pt[:, j, :], ps[:], AF.Exp,
                        bias=expb[:], scale=float(2.0 ** -13),
                    )
                if fill is not None:
                    # overlap work goes between the scores and the ctx
                    # matmuls: the PE would otherwise idle here waiting on
                    # the exps that the ctx matmuls consume
                    fill()
                for hh, pc in ((0, pc0), (1, pc1)):
                    h = 2 * hp + hh
                    nc.tensor.matmul(
                        pc[:],
                        vo5[:, ktp, :, h, 0 : DK + 1],
                        pt[:, :, hh * 512 : hh * 512 + 512],
                        start=(ktp == 0),
                        stop=(ktp == NSP - 1),
                        perf_mode=DR,
                    )

            def chain(hp, qc):
                # softmax normalize: ctxT = 32*ctx in fp8
                pc0, pc1 = pcs
                for hh, pc in ((0, pc0), (1, pc1)):
                    dcp = up.tile([1, 512], f32, tag="dcp")
                    nc.vector.tensor_copy(dcp[:], pc[DK : DK + 1, :])
                    rcp = up.tile([1, 512], f32, tag="rcp")
                    nc.vector.reciprocal_approx_fast(rcp[:], dcp[:])
                    rb = up.tile([DK, 512], f32, tag="rb")
                    nc.gpsimd.partition_broadcast(rb[:], rcp[:])
                    nc.vector.tensor_tensor(
                        ctxT[
                            hh * DK : hh * DK + DK, hp,
                            qc * 512 : (qc + 1) * 512,
                        ],
                        pc[0:DK, :],
                        rb[:],
                        OP.mult,
                    )

            # =================== attention + overlapped C/FFN ===============
            with (
                tc.tile_pool(name="xtp", bufs=1) as xtp,
                tc.tile_pool(name="wp", bufs=1) as wp,
            ):
                xt = xtp.tile([P, NE, S], fp8)
                wv_sb = wp.tile([P, NE, D], fp8, tag="wv")
                for et in range(NE):
                    nc.sync.dma_start(
                        wv_sb[:, et, :], wv_d.ap()[et * P : (et + 1) * P, :]
                    )
                    nc.sync.dma_start(
                        xt[:, et, :], xt_d.ap()[et * P : (et + 1) * P, :]
                    )
                wq_sb = wp.tile([P, NE, D], fp8, tag="wq")
                wk_sb = wp.tile([P, NE, D], fp8, tag="wk")
                for et in range(NE):
                    nc.sync.dma_start(
                        wq_sb[:, et, :], wq_d.ap()[et * P : (et + 1) * P, :]
                    )
                    nc.sync.dma_start(
                        wk_sb[:, et, :], wk_d.ap()[et * P : (et + 1) * P, :]
                    )

                # ones columns of V' (1.0; with wv x32 and pt=p/16 the
                # normalize yields ctxT = 32*ctx)
                ones192 = xtp.tile([P, NS * H], f32, tag="ones192")
                nc.gpsimd.memset(ones192[:], 1.0)
                nc.vector.tensor_copy(
                    vo5[:, :, :, :, DK : DK + 1],
                    ones192[:].rearrange(
                        "p (s j h o) -> p s j h o", s=NSP, j=2, h=H
                    ),
                )

                def v_chunk(st):
                    for dc, cw in ((0, 512), (512, 256)):
                        ps = psX.tile([P, 512], f32, tag="x")
                        for ep in range(NEP):
                            nc.tensor.matmul(
                                ps[:, 0:cw],
                                xt[:, 2 * ep : 2 * ep + 2, st * P : (st + 1) * P],
                                wv_sb[:, 2 * ep : 2 * ep + 2, dc : dc + cw],
                                start=(ep == 0),
                                stop=(ep == NEP - 1),
                                perf_mode=DR,
                            )
                        h0, nh = dc // DK, cw // DK
                        nc.vector.tensor_copy(
                            vo5[:, st // 2, st % 2, h0 : h0 + nh, 0:DK],
                            ps[:, 0:cw].rearrange("p (h w) -> p h w", w=DK),
                        )

                def q_proj(hp):
                    qh = kqr.tile([P, QR], fp8, tag="qh")
                    qh_t[hp] = qh
                    for qc in range(QR // 512):
                        ps = psX.tile([P, 512], f32, tag="x")
                        for ep in range(NEP):
                            nc.tensor.matmul(
                                ps[:],
                                wq_sb[:, 2 * ep : 2 * ep + 2, hp * P : (hp + 1) * P],
                                xt[:, 2 * ep : 2 * ep + 2, qc * 512 : (qc + 1) * 512],
                                start=(ep == 0),
                                stop=(ep == NEP - 1),
                                perf_mode=DR,
                            )
                        nc.vector.tensor_copy(qh[:, qc * 512 : (qc + 1) * 512], ps[:])

                def k_proj(hp):
                    kh0 = kqr.tile([P, S], fp8, tag="kh0")
                    kh1 = kqr.tile([P, S], fp8, tag="kh1")
                    kh_t[hp] = (kh0, kh1)
                    nc.gpsimd.memset(kh0[DK:P, :], 0.0)
                    nc.gpsimd.memset(kh1[0:DK, :], 0.0)
                    for sc in range(S // 512):
                        ps = psX.tile([P, 512], f32, tag="x")
                        for ep in range(NEP):
                            nc.tensor.matmul(
                                ps[:],
                                wk_sb[:, 2 * ep : 2 * ep + 2, hp * P : (hp + 1) * P],
                                xt[:, 2 * ep : 2 * ep + 2, sc * 512 : (sc + 1) * 512],
                                start=(ep == 0),
                                stop=(ep == NEP - 1),
                                perf_mode=DR,
                            )
                        nc.vector.tensor_copy(
                            kh0[0:DK, sc * 512 : (sc + 1) * 512], ps[0:DK, :]
                        )
                        nc.vector.tensor_copy(
                            kh1[DK:P, sc * 512 : (sc + 1) * 512], ps[DK:P, :]
                        )

                q_proj(0)
                k_proj(0)
                for et in range(NE):
                    nc.sync.dma_start(
                        wo_sb[:, et, :], wo_d.ap()[et * P : (et + 1) * P, :]
                    )

                # ------------- qc = 0 pass (V-proj + Q/K fillers) ----------
                for hp in range(H // 2):
                    pc0 = psC.tile([DK + 1, 512], f32, tag="c0")
                    pc1 = psC.tile([DK + 1, 512], f32, tag="c1")
                    pcs[0], pcs[1] = pc0, pc1
                    for ktp in range(NSP):
                        if hp == 0:
                            v_chunk(2 * ktp)
                            v_chunk(2 * ktp + 1)
                        kt_body(hp, 0, ktp)
                    if hp + 1 < H // 2:
                        q_proj(hp + 1)
                        k_proj(hp + 1)
                    chain(hp, 0)
            # xt / wq / wk / wv freed here

            # residual rows for the first half + w1 chunk prefetch
            xh_sb0 = xhp.tile([P, 4, D], bf16, tag="xh")
            for qt_i in range(4):
                nc.sync.dma_start(
                    xh_sb0[:, qt_i, :], xh_d.ap()[qt_i * P : (qt_i + 1) * P, :]
                )
            w1_dma(0, 0)
            w1_dma(0, 1)

            # work items hidden inside the qc=1 attention pass; transposes
            # trail their LN by two slots to hide the DVE chain latency
            items = (
                [("ca", 0), ("ca", 1), ("cb", 0), ("ca", 2), ("cb", 1),
                 ("ca", 3), ("cb", 2), ("cb", 3)]
                + [("w1", f) for f in range(NF)]
            )
            it = [0]

            def emit_item():
                if it[0] >= len(items):
                    return
                kind, a = items[it[0]]
                it[0] += 1
                if kind == "ca":
                    c_item_a(a, xh_sb0, 0)
                elif kind == "cb":
                    c_item_b(a)
                elif kind == "w1":
                    w1_item(0, a)

            # ------------------------- qc = 1 pass ---------------------------
            for hp in range(H // 2):
                pc0 = psC.tile([DK + 1, 512], f32, tag="c0")
                pc1 = psC.tile([DK + 1, 512], f32, tag="c1")
                pcs[0], pcs[1] = pc0, pc1
                for ktp in range(NSP):
                    kt_body(hp, 1, ktp, fill=emit_item)
                chain(hp, 1)
            while it[0] < len(items):
                emit_item()
            es_ab.close()  # free vo / qh / kh / pt / scores+ctx psum

            # ------------------------------ tail -----------------------------
            with (
                tc.tile_pool(name="w2p", bufs=1) as w2p,
                tc.tile_pool(name="sp2", bufs=3) as sp2,
                tc.tile_pool(name="psF", bufs=4, space="PSUM") as psF,
            ):
                w2_sb = w2p.tile([P, NF, D], bf16)
                for fc in range(NF // 4):
                    nc.sync.dma_start(
                        w2_sb[:, fc * 4 : (fc + 1) * 4, :],
                        dram3(w2_d.ap()[fc * 4 * P : (fc + 1) * 4 * P, :]),
                    )
                xh_sb1 = xhp.tile([P, 4, D], bf16, tag="xh")
                for qt_i in range(4):
                    nc.sync.dma_start(
                        xh_sb1[:, qt_i, :],
                        xh_d.ap()[(qt_i + 4) * P : (qt_i + 5) * P, :],
                    )
                w1_dma(1, 0)
                w1_dma(1, 1)
                # LN chains use the tail pool (bufs=3) and the transposes
                # trail their LN by one item so the PE never waits on DVE
                c_item_a(4, xh_sb1, 4, sp2)
                c_item_a(5, xh_sb1, 4, sp2)
                c_item_b(4)
                c_item_a(6, xh_sb1, 4, sp2)
                c_item_b(5)
                c_item_a(7, xh_sb1, 4, sp2)
                c_item_b(6)
                c_item_b(7)
                for f_t in range(NF):
                    w1_item(1, f_t)

                def w2_block(qt_i):
                    qc = qt_i // 4
                    ht = ht_t[qc]
                    t2 = sp2.tile([P, D], f32, tag="t2")
                    for dc, cw in ((0, 512), (512, 256)):
                        ps = psF.tile([P, 512], f32, tag="psF")
                        for f_t in range(NF):
                            nc.tensor.matmul(
                                ps[:, 0:cw],
                                ht[:, f_t, (qt_i % 4) * P : (qt_i % 4 + 1) * P],
                                w2_sb[:, f_t, dc : dc + cw],
                                start=(f_t == 0),
                                stop=False,
                            )
                        nc.tensor.matmul(
                            ps[:, 0:cw],
                            ones_bf[0:1, 0:P],
                            b2_sb[0:1, dc : dc + cw],
                            start=False,
                            stop=True,
                        )
                        nc.vector.tensor_add(
                            t2[:, dc : dc + cw], x1[:, qt_i, dc : dc + cw],
                            ps[:, 0:cw],
                        )
                    osb = sp2.tile([P, D], f32, tag="osb")
                    layer_norm(t2[:], osb[:], l2a, l2b, sp2)
                    nc.sync.dma_start(
                        out_d.ap()[qt_i * P : (qt_i + 1) * P, :], osb[:]
                    )

                for qt_i in range(4):
                    w2_block(qt_i)
                for qt_i in range(4, NQ):
                    w2_block(qt_i)

    nc.compile()
    return nc


def _prep_in_maps(inputs):
    import ml_dtypes

    fp8 = ml_dtypes.float8_e4m3

    x = np.asarray(inputs["x"], dtype=np.float32)
    wq = np.ascontiguousarray(
        (np.asarray(inputs["wq"], np.float32) * 32.0).astype(fp8)
    )
    wk = np.ascontiguousarray(
        (np.asarray(inputs["wk"], np.float32) * 32.0).astype(fp8)
    )
    wv = np.ascontiguousarray(
        (np.asarray(inputs["wv"], np.float32) * 32.0).astype(fp8)
    )
    wo = np.ascontiguousarray(
        (np.asarray(inputs["wo"], np.float32) * 64.0).astype(fp8)
    )
    w1 = np.ascontiguousarray(
        np.asarray(inputs["w1"], np.float32).astype(ml_dtypes.bfloat16)
    )
    w2 = np.ascontiguousarray(
        np.asarray(inputs["w2"], np.float32).astype(ml_dtypes.bfloat16)
    )
    b1t = np.ascontiguousarray(
        np.asarray(inputs["b1"], np.float32).reshape(NF, P).T
    )
    b2r = np.ascontiguousarray(
        np.asarray(inputs["b2"], np.float32).reshape(1, D).astype(ml_dtypes.bfloat16)
    )
    ln1a = np.ascontiguousarray(
        np.broadcast_to(np.asarray(inputs["ln1_alpha"], np.float32), (P, D))
    )
    ln1b = np.ascontiguousarray(
        np.broadcast_to(np.asarray(inputs["ln1_bias"], np.float32), (P, D))
    )
    ln2a = np.ascontiguousarray(
        np.broadcast_to(np.asarray(inputs["ln2_alpha"], np.float32), (P, D))
    )
    ln2b = np.ascontiguousarray(
        np.broadcast_to(np.asarray(inputs["ln2_bias"], np.float32), (P, D))
    )
    shared = dict(
        wq=wq, wk=wk, wv=wv, wo=wo, w1=w1, w2=w2,
        b1t=b1t, b2r=b2r, ln1a=ln1a, ln1b=ln1b, ln2a=ln2a, ln2b=ln2b,
    )
    in_maps = []
    for c in range(NCORES):
        b, half = c // 2, c % 2
        xb = x[b]  # [S, D]
        rolled = np.concatenate([xb[half * QR :], xb[: half * QR]], axis=0)
        m = dict(shared)
        m["xt"] = np.ascontiguousarray(rolled.T.astype(fp8))
        m["xh"] = np.ascontiguousarray(
            xb[half * QR : half * QR + QR].astype(ml_dtypes.bfloat16)
        )
        in_maps.append(m)
    return in_maps


def _skip_affine(inputs):
    return (
        np.all(np.asarray(inputs["ln1_alpha"]) == 1.0)
        and np.all(np.asarray(inputs["ln2_alpha"]) == 1.0)
        and np.all(np.asarray(inputs["ln1_bias"]) == 0.0)
        and np.all(np.asarray(inputs["ln2_bias"]) == 0.0)
    )


def kernel(**inputs):
    from concourse.bass_utils import run_bass_kernel_spmd

    sa = bool(_skip_affine(inputs))
    key = ("nc", sa)
    if key not in _CACHE:
        _CACHE[key] = _build(sa)
    nc = _CACHE[key]
    in_maps = _prep_in_maps(inputs)
    res = run_bass_kernel_spmd(nc, in_maps, core_ids=list(range(NCORES)))
    out = np.empty((B, S, D), dtype=np.float32)
    for c in range(NCORES):
        b, half = c // 2, c % 2
        out[b, half * QR : half * QR + QR, :] = res.results[c]["out"]
    return out


# revision 45
# speedup vs baseline: 1.0565x; 1.0124x over previous
"""Trainium2 Bass kernel for a dense transformer encoder block.

Sharding: pure data-parallel, zero collectives. 8 cores; core c handles
batch b = c//2, query rows half = c%2 (1024 of 2048 seq positions).
Each core receives the full (sequence-rotated) x[b]^T so it can compute
K/V over all 2048 keys locally; queries are always columns 0:1024 of the
rotated x^T (attention is permutation-invariant over the key axis).

v3: query-chunk-outer attention with software-pipelined overlap: during
the second query-chunk's attention pass (which is paced by the Scalar
engine's softmax exps), the PE queue is fed the first chunk's wo
projection + LN1 + transposes and its full w1 FFN layer, one work item
per kt-pair slot.  fp8e4 DoubleRow matmuls (2 k-tiles per instruction)
for the V/Q/K projections and the attention*V matmul; fp8 scores
operands and fp8 ctxT/wo.  All fp8 prescales are powers of two and are
compensated exactly:
  - wq,wk x32 -> qh/kh store 32q/32k in fp8; scores psum = 1024*(q.k);
    exp runs with scale=2^-13 and bias=-ln16 (pt = p/16, max ~42 << 240).
  - wv x32, ones-column of V' = 1.0 -> ctx psum rows = 2*sum(p)v, denom
    row = sum(p)/16; reciprocal-normalize yields ctxT = 32*ctx in fp8.
  - wo x64 in fp8 -> wo psum = 2048*attn_out; one fused DVE op computes
    psum*2^-11 + x residual.
Scores run K=128 against zero-padded kh tiles (no PE tiling-mode
switches, keeps ctx DoubleRow weight-loads hidden).  w1 streams in
double-buffered chunks; w2 is resident only for the tail.
"""

import sys

if "/opt/trn_rl_repo" not in sys.path:
    sys.path.insert(0, "/opt/trn_rl_repo")

import numpy as np

B, S, D, H, DK, DFF = 4, 2048, 768, 12, 64, 3072
NCORES = 8
QR = 1024  # query rows per core
EPS = 1e-6
P = 128
NE = D // P  # 6 e-tiles (contraction over model dim)
NEP = NE // 2  # 3 DoubleRow pairs
NS = S // P  # 16 s-tiles (key positions)
NSP = NS // 2  # 8 DoubleRow kt pairs
NQ = QR // P  # 8 q-tiles
NF = DFF // P  # 24 f-tiles
HW = 80  # padded per-head width in V' (64 d + ones col + pad)
LN16 = 2.772588722239781  # ln(16)

_CACHE = {}


def _build(skip_affine, b2_zero=False):
    from contextlib import ExitStack

    import concourse.bass as bass
    import concourse.tile as tile
    from concourse import bacc, mybir
    from concourse.masks import make_identity

    dt = mybir.dt
    f32 = dt.float32
    bf16 = dt.bfloat16
    fp8 = dt.float8e4
    AF = mybir.ActivationFunctionType
    OP = mybir.AluOpType
    DR = mybir.MatmulPerfMode.DoubleRow

    nc = bacc.Bacc("TRN2", target_bir_lowering=False, debug=False)

    xt_d = nc.dram_tensor("xt", [D, S], fp8, kind="ExternalInput")
    xh_d = nc.dram_tensor("xh", [QR, D], bf16, kind="ExternalInput")
    wq_d = nc.dram_tensor("wq", [D, D], fp8, kind="ExternalInput")  # x32
    wk_d = nc.dram_tensor("wk", [D, D], fp8, kind="ExternalInput")  # x32
    wv_d = nc.dram_tensor("wv", [D, D], fp8, kind="ExternalInput")  # x32
    wo_d = nc.dram_tensor("wo", [D, D], fp8, kind="ExternalInput")  # x64
    w1_d = nc.dram_tensor("w1", [D, DFF], bf16, kind="ExternalInput")
    w2_d = nc.dram_tensor("w2", [DFF, D], bf16, kind="ExternalInput")
    b1_d = nc.dram_tensor("b1t", [P, NF], f32, kind="ExternalInput")  # b1 T'd
    b2_d = nc.dram_tensor("b2r", [1, D], bf16, kind="ExternalInput")
    ln1a_d = nc.dram_tensor("ln1a", [P, D], f32, kind="ExternalInput")  # bcast
    ln1b_d = nc.dram_tensor("ln1b", [P, D], f32, kind="ExternalInput")
    ln2a_d = nc.dram_tensor("ln2a", [P, D], f32, kind="ExternalInput")
    ln2b_d = nc.dram_tensor("ln2b", [P, D], f32, kind="ExternalInput")
    out_d = nc.dram_tensor("out", [QR, D], f32, kind="ExternalOutput")

    def dram3(d_ap, p=P):
        return d_ap.rearrange("(n p) s -> p n s", p=p)

    with tile.TileContext(nc) as tc:
        with ExitStack() as ctx:
            const = ctx.enter_context(tc.tile_pool(name="const", bufs=1))
            ones_bf = const.tile([1, P], bf16)
            nc.gpsimd.memset(ones_bf[:], 1.0)
            ident = const.tile([P, P], f32)
            make_identity(nc, ident[:])
            b1_sb = const.tile([P, NF], f32)
            nc.sync.dma_start(b1_sb[:], b1_d.ap())
            b2_sb = const.tile([1, D], bf16)
            nc.sync.dma_start(b2_sb[:], b2_d.ap())
            expb = const.tile([P, 1], f32)  # exp bias: -ln(16)
            nc.gpsimd.memset(expb[:], -LN16)

            # persistent (whole kernel) left-side pools
            xp = ctx.enter_context(tc.tile_pool(name="xp", bufs=1))
            x1 = xp.tile([P, NQ, D], f32, tag="x1")
            x1t = xp.tile([P, NE, QR], bf16, tag="x1t")
            xhp = ctx.enter_context(tc.tile_pool(name="xhp", bufs=1))
            spL = ctx.enter_context(tc.tile_pool(name="spL", bufs=1))
            lnc = ctx.enter_context(tc.tile_pool(name="lnc", bufs=1))
            if skip_affine:
                l1a = l1b = l2a = l2b = None
            else:
                l1a = lnc.tile([P, D], f32, tag="l1a")
                l1b = lnc.tile([P, D], f32, tag="l1b")
                l2a = lnc.tile([P, D], f32, tag="l2a")
                l2b = lnc.tile([P, D], f32, tag="l2b")
                nc.sync.dma_start(l1a[:], ln1a_d.ap())
                nc.sync.dma_start(l1b[:], ln1b_d.ap())
                nc.sync.dma_start(l2a[:], ln2a_d.ap())
                nc.sync.dma_start(l2b[:], ln2b_d.ap())
            w1p = ctx.enter_context(tc.tile_pool(name="w1p", bufs=2))
            # bufs=2: ht(qc1) relu must not wait on ht(qc0)'s w2 readers,
            # which sit later in the PE queue (would deadlock at bufs=1)
            htp = ctx.enter_context(tc.tile_pool(name="htp", bufs=2))

            # right side: ctx^T (fp8, = 32*ctx) + wo (fp8, = 64*wo)
            ctp = ctx.enter_context(tc.tile_pool(name="ctp", bufs=1, side="right"))
            ctxT = ctp.tile([P, NE, QR], fp8, tag="ctxT")
            wo_sb = ctp.tile([P, NE, D], fp8, tag="wo")

            psX = ctx.enter_context(tc.tile_pool(name="psX", bufs=2, space="PSUM"))

            es_ab = ExitStack()  # attention-lifetime pools
            kqv = es_ab.enter_context(tc.tile_pool(name="kqv", bufs=1))
            vo = kqv.tile([P, NSP, 2, H * HW], fp8, tag="vo")
            vo5 = vo[:, :, :, :].rearrange("p s j (h w) -> p s j h w", w=HW)
            kqr = es_ab.enter_context(tc.tile_pool(name="kqr", bufs=6))
            ptp = es_ab.enter_context(tc.tile_pool(name="ptp", bufs=2))
            up = es_ab.enter_context(tc.tile_pool(name="up", bufs=2))
            psS = es_ab.enter_context(tc.tile_pool(name="psS", bufs=2, space="PSUM"))
            psC = es_ab.enter_context(tc.tile_pool(name="psC", bufs=1, space="PSUM"))

            def layer_norm(tin, out_ap, a_bc, b_bc, spool):
                st6 = spool.tile([P, 2, 6], f32, tag="st6")
                nc.vector.bn_stats(st6[:, 0, :], tin[:, 0:384])
                nc.vector.bn_stats(st6[:, 1, :], tin[:, 384:768])
                mv = spool.tile([P, 2], f32, tag="mv")
                nc.vector.bn_aggr(mv[:], st6[:])
                std = spool.tile([P, 1], f32, tag="std")
                nc.scalar.activation(
                    std[:], mv[:, 1:2], AF.Sqrt, scale=float(D) / (D - 1)
                )
                stde = spool.tile([P, 1], f32, tag="stde")
                nc.vector.tensor_scalar_add(stde[:], std[:], EPS)
                rstd = spool.tile([P, 1], f32, tag="rstd")
                nc.vector.reciprocal(rstd[:], stde[:])
                if skip_affine:
                    nc.vector.tensor_scalar(
                        out_ap, tin[:], mv[:, 0:1], rstd[:],
                        op0=OP.subtract, op1=OP.mult,
                    )
                else:
                    yc = spool.tile([P, D], f32, tag="yc")
                    nc.vector.tensor_scalar(
                        yc[:], tin[:], mv[:, 0:1], rstd[:],
                        op0=OP.subtract, op1=OP.mult,
                    )
                    y2 = spool.tile([P, D], f32, tag="y2")
                    nc.vector.tensor_tensor(y2[:], yc[:], a_bc, OP.mult)
                    nc.vector.tensor_tensor(out_ap, y2[:], b_bc, OP.add)

            w1tiles = {}

            def w1_dma(qc, c):
                t = w1p.tile([P, NE, 4 * P], bf16, tag="w1c")
                for et in range(NE):
                    nc.sync.dma_start(
                        t[:, et, :],
                        w1_d.ap()[
                            et * P : (et + 1) * P, c * 4 * P : (c + 1) * 4 * P
                        ],
                    )
                w1tiles[(qc, c)] = t

            ht_t = [None, None]

            def w1_item(qc, f_t):
                c = f_t // 4
                if f_t % 4 == 0 and c + 2 < NF // 4:
                    w1_dma(qc, c + 2)
                if f_t == 0:
                    ht_new = htp.tile([P, NF, 512], bf16, tag="ht")
                    ht_t[qc] = ht_new
                ht = ht_t[qc]
                w1c = w1tiles[(qc, c)]
                ps = psX.tile([P, 512], f32, tag="x")
                for et in range(NE):
                    nc.tensor.matmul(
                        ps[:],
                        w1c[:, et, (f_t % 4) * P : (f_t % 4 + 1) * P],
                        x1t[:, et, qc * 512 : (qc + 1) * 512],
                        start=(et == 0),
                        stop=(et == NE - 1),
                    )
                nc.scalar.activation(
                    ht[:, f_t, :], ps[:], AF.Relu, bias=b1_sb[:, f_t : f_t + 1]
                )

            def c_item_a(qt_i, xh_sb, xo, spool=None):
                # wo projection (fp8: psum = 2048*attn) + residual + LN1
                spool = spool if spool is not None else spL
                tsb = spool.tile([P, D], f32, tag="tsb")
                for dc, cw in ((0, 512), (512, 256)):
                    ps = psX.tile([P, 512], f32, tag="x")
                    for dt_i in range(NE):
                        nc.tensor.matmul(
                            ps[:, 0:cw],
                            ctxT[:, dt_i, qt_i * P : (qt_i + 1) * P],
                            wo_sb[:, dt_i, dc : dc + cw],
                            start=(dt_i == 0),
                            stop=(dt_i == NE - 1),
                        )
                    nc.vector.scalar_tensor_tensor(
                        tsb[:, dc : dc + cw],
                        ps[:, 0:cw],
                        float(2.0 ** -11),
                        xh_sb[:, qt_i - xo, dc : dc + cw],
                        op0=OP.mult,
                        op1=OP.add,
                    )
                layer_norm(tsb[:], x1[:, qt_i, :], l1a, l1b, spool)

            def c_item_b(qt_i):
                # x1^T transposes, emitted a couple of slots after c_item_a
                # so the PE never waits on the LN chain
                for dt_i in range(NE):
                    pst = psX.tile([P, 512], f32, tag="x")
                    nc.tensor.transpose(
                        pst[:, 0:P], x1[:, qt_i, dt_i * P : (dt_i + 1) * P],
                        ident[:],
                    )
                    nc.vector.tensor_copy(
                        x1t[:, dt_i, qt_i * P : (qt_i + 1) * P], pst[:, 0:P]
                    )

            def c_item(qt_i, xh_sb, xo):
                c_item_a(qt_i, xh_sb, xo)
                c_item_b(qt_i)

            qh_t = [None] * (H // 2)
            kh_t = [None] * (H // 2)
            pcs = [None, None]

            def kt_body(hp, qc, ktp, fill=None):
                qh = qh_t[hp]
                kh0, kh1 = kh_t[hp]
                pc0, pc1 = pcs
                pt = ptp.tile([P, 2, 1024], fp8, tag="pt")
                for j in range(2):
                    kt_i = 2 * ktp + j
                    ps = psS.tile([P, 1024], f32, tag="psS")
                    for hh, khp in ((0, kh0), (1, kh1)):
                        nc.tensor.matmul(
                            ps[:, hh * 512 : hh * 512 + 512],
                            khp[:, kt_i * P : (kt_i + 1) * P],
                            qh[:, qc * 512 : (qc + 1) * 512],
                            start=True,
                            stop=True,
                        )
                    # pt = exp(scores)/16; scores psum = 1024*(q.k)
                    nc.scalar.activation(
                        pt[:, j, :], ps[:], AF.Exp,
                        bias=expb[:], scale=float(2.0 ** -13),
                    )
                if fill is not None:
                    # overlap work goes between the scores and the ctx
                    # matmuls: the PE would otherwise idle here waiting on
                    # the exps that the ctx matmuls consume
                    fill()
                for hh, pc in ((0, pc0), (1, pc1)):
                    h = 2 * hp + hh
                    nc.tensor.matmul(
                        pc[:],
                        vo5[:, ktp, :, h, 0 : DK + 1],
                        pt[:, :, hh * 512 : hh * 512 + 512],
                        start=(ktp == 0),
                        stop=(ktp == NSP - 1),
                        perf_mode=DR,
                    )

            def chain(hp, qc):
                # softmax normalize: ctxT = 32*ctx in fp8
                pc0, pc1 = pcs
                for hh, pc in ((0, pc0), (1, pc1)):
                    dcp = up.tile([1, 512], f32, tag="dcp")
                    nc.vector.tensor_copy(dcp[:], pc[DK : DK + 1, :])
                    rcp = up.tile([1, 512], f32, tag="rcp")
                    nc.vector.reciprocal_approx_fast(rcp[:], dcp[:])
                    rb = up.tile([DK, 512], f32, tag="rb")
                    nc.gpsimd.partition_broadcast(rb[:], rcp[:])
                    nc.vector.tensor_tensor(
                        ctxT[
                            hh * DK : hh * DK + DK, hp,
                            qc * 512 : (qc + 1) * 512,
                        ],
                        pc[0:DK, :],
                        rb[:],
                        OP.mult,
                    )

            # =================== attention + overlapped C/FFN ===============
            with (
                tc.tile_pool(name="xtp", bufs=1) as xtp,
                tc.tile_pool(name="wp", bufs=1) as wp,
            ):
                # DMA order = first-use order: the first PE work is
                # q_proj(0)/k_proj(0) (xt + wq/wk), then V chunks (wv)
                xt = xtp.tile([P, NE, S], fp8)
                wv_sb = wp.tile([P, NE, D], fp8, tag="wv")
                wq_sb = wp.tile([P, NE, D], fp8, tag="wq")
                wk_sb = wp.tile([P, NE, D], fp8, tag="wk")
                for et in range(NE):
                    nc.sync.dma_start(
                        xt[:, et, :], xt_d.ap()[et * P : (et + 1) * P, :]
                    )
                for et in range(NE):
                    nc.sync.dma_start(
                        wq_sb[:, et, :], wq_d.ap()[et * P : (et + 1) * P, :]
                    )
                    nc.sync.dma_start(
                        wk_sb[:, et, :], wk_d.ap()[et * P : (et + 1) * P, :]
                    )
                for et in range(NE):
                    nc.sync.dma_start(
                        wv_sb[:, et, :], wv_d.ap()[et * P : (et + 1) * P, :]
                    )

                # ones columns of V' (1.0; with wv x32 and pt=p/16 the
                # normalize yields ctxT = 32*ctx)
                ones192 = xtp.tile([P, NS * H], f32, tag="ones192")
                nc.gpsimd.memset(ones192[:], 1.0)
                nc.vector.tensor_copy(
                    vo5[:, :, :, :, DK : DK + 1],
                    ones192[:].rearrange(
                        "p (s j h o) -> p s j h o", s=NSP, j=2, h=H
                    ),
                )

                def v_chunk(st):
                    for dc, cw in ((0, 512), (512, 256)):
                        ps = psX.tile([P, 512], f32, tag="x")
                        for ep in range(NEP):
                            nc.tensor.matmul(
                                ps[:, 0:cw],
                                xt[:, 2 * ep : 2 * ep + 2, st * P : (st + 1) * P],
                                wv_sb[:, 2 * ep : 2 * ep + 2, dc : dc + cw],
                                start=(ep == 0),
                                stop=(ep == NEP - 1),
                                perf_mode=DR,
                            )
                        h0, nh = dc // DK, cw // DK
                        nc.vector.tensor_copy(
                            vo5[:, st // 2, st % 2, h0 : h0 + nh, 0:DK],
                            ps[:, 0:cw].rearrange("p (h w) -> p h w", w=DK),
                        )

                def q_proj(hp):
                    qh = kqr.tile([P, QR], fp8, tag="qh")
                    qh_t[hp] = qh
                    for qc in range(QR // 512):
                        ps = psX.tile([P, 512], f32, tag="x")
                        for ep in range(NEP):
                            nc.tensor.matmul(
                                ps[:],
                                wq_sb[:, 2 * ep : 2 * ep + 2, hp * P : (hp + 1) * P],
                                xt[:, 2 * ep : 2 * ep + 2, qc * 512 : (qc + 1) * 512],
                                start=(ep == 0),
                                stop=(ep == NEP - 1),
                                perf_mode=DR,
                            )
                        nc.vector.tensor_copy(qh[:, qc * 512 : (qc + 1) * 512], ps[:])

                def k_proj(hp):
                    kh0 = kqr.tile([P, S], fp8, tag="kh0")
                    kh1 = kqr.tile([P, S], fp8, tag="kh1")
                    kh_t[hp] = (kh0, kh1)
                    nc.gpsimd.memset(kh0[DK:P, :], 0.0)
                    nc.gpsimd.memset(kh1[0:DK, :], 0.0)
                    for sc in range(S // 512):
                        ps = psX.tile([P, 512], f32, tag="x")
                        for ep in range(NEP):
                            nc.tensor.matmul(
                                ps[:],
                                wk_sb[:, 2 * ep : 2 * ep + 2, hp * P : (hp + 1) * P],
                                xt[:, 2 * ep : 2 * ep + 2, sc * 512 : (sc + 1) * 512],
                                start=(ep == 0),
                                stop=(ep == NEP - 1),
                                perf_mode=DR,
                            )
                        nc.vector.tensor_copy(
                            kh0[0:DK, sc * 512 : (sc + 1) * 512], ps[0:DK, :]
                        )
                        nc.vector.tensor_copy(
                            kh1[DK:P, sc * 512 : (sc + 1) * 512], ps[DK:P, :]
                        )

                q_proj(0)
                k_proj(0)
                for et in range(NE):
                    nc.sync.dma_start(
                        wo_sb[:, et, :], wo_d.ap()[et * P : (et + 1) * P, :]
                    )

                # ------------- qc = 0 pass (V-proj + Q/K fillers) ----------
                for hp in range(H // 2):
                    pc0 = psC.tile([DK + 1, 512], f32, tag="c0")
                    pc1 = psC.tile([DK + 1, 512], f32, tag="c1")
                    pcs[0], pcs[1] = pc0, pc1
                    for ktp in range(NSP):
                        if hp == 0:
                            v_chunk(2 * ktp)
                            v_chunk(2 * ktp + 1)
                        kt_body(hp, 0, ktp)
                    if hp + 1 < H // 2:
                        q_proj(hp + 1)
                        k_proj(hp + 1)
                    chain(hp, 0)
            # xt / wq / wk / wv freed here

            # residual rows for the first half + w1 chunk prefetch
            xh_sb0 = xhp.tile([P, 4, D], bf16, tag="xh")
            for qt_i in range(4):
                nc.sync.dma_start(
                    xh_sb0[:, qt_i, :], xh_d.ap()[qt_i * P : (qt_i + 1) * P, :]
                )
            w1_dma(0, 0)
            w1_dma(0, 1)

            # work items hidden inside the qc=1 attention pass; transposes
            # trail their LN by two slots to hide the DVE chain latency
            items = (
                [("ca", 0), ("ca", 1), ("cb", 0), ("ca", 2), ("cb", 1),
                 ("ca", 3), ("cb", 2), ("cb", 3)]
                + [("w1", f) for f in range(NF)]
            )
            it = [0]

            def emit_item():
                if it[0] >= len(items):
                    return
                kind, a = items[it[0]]
                it[0] += 1
                if kind == "ca":
                    c_item_a(a, xh_sb0, 0)
                elif kind == "cb":
                    c_item_b(a)
                elif kind == "w1":
                    w1_item(0, a)

            # ------------------------- qc = 1 pass ---------------------------
            for hp in range(H // 2):
                pc0 = psC.tile([DK + 1, 512], f32, tag="c0")
                pc1 = psC.tile([DK + 1, 512], f32, tag="c1")
                pcs[0], pcs[1] = pc0, pc1
                for ktp in range(NSP):
                    kt_body(hp, 1, ktp, fill=emit_item)
                chain(hp, 1)
            while it[0] < len(items):
                emit_item()
            # prefetch the tail's residual rows before the phase boundary
            xh_sb1 = xhp.tile([P, 4, D], bf16, tag="xh")
            for qt_i in range(4):
                nc.sync.dma_start(
                    xh_sb1[:, qt_i, :],
                    xh_d.ap()[(qt_i + 4) * P : (qt_i + 5) * P, :],
                )
            es_ab.close()  # free vo / qh / kh / pt / scores+ctx psum

            # ------------------------------ tail -----------------------------
            with (
                tc.tile_pool(name="w2p", bufs=1) as w2p,
                tc.tile_pool(name="sp2", bufs=3) as sp2,
                tc.tile_pool(name="psF", bufs=4, space="PSUM") as psF,
            ):
                w2_sb = w2p.tile([P, NF, D], bf16)
                for fc in range(NF // 4):
                    nc.sync.dma_start(
                        w2_sb[:, fc * 4 : (fc + 1) * 4, :],
                        dram3(w2_d.ap()[fc * 4 * P : (fc + 1) * 4 * P, :]),
                    )
                w1_dma(1, 0)
                w1_dma(1, 1)
                # LN chains use the tail pool (bufs=3) and the transposes
                # trail their LN by one item so the PE never waits on DVE
                c_item_a(4, xh_sb1, 4, sp2)
                c_item_a(5, xh_sb1, 4, sp2)
                c_item_b(4)
                c_item_a(6, xh_sb1, 4, sp2)
                c_item_b(5)
                c_item_a(7, xh_sb1, 4, sp2)
                c_item_b(6)
                c_item_b(7)
                for f_t in range(NF):
                    w1_item(1, f_t)

                def w2_block(qt_i):
                    qc = qt_i // 4
                    ht = ht_t[qc]
                    t2 = sp2.tile([P, D], f32, tag="t2")
                    for dc, cw in ((0, 512), (512, 256)):
                        ps = psF.tile([P, 512], f32, tag="psF")
                        for f_t in range(NF):
                            nc.tensor.matmul(
                                ps[:, 0:cw],
                                ht[:, f_t, (qt_i % 4) * P : (qt_i % 4 + 1) * P],
                                w2_sb[:, f_t, dc : dc + cw],
                                start=(f_t == 0),
                                stop=(b2_zero and f_t == NF - 1),
                            )
                        if not b2_zero:
                            nc.tensor.matmul(
                                ps[:, 0:cw],
                                ones_bf[0:1, 0:P],
                                b2_sb[0:1, dc : dc + cw],
                                start=False,
                                stop=True,
                            )
                        nc.vector.tensor_add(
                            t2[:, dc : dc + cw], x1[:, qt_i, dc : dc + cw],
                            ps[:, 0:cw],
                        )
                    osb = sp2.tile([P, D], f32, tag="osb")
                    layer_norm(t2[:], osb[:], l2a, l2b, sp2)
                    nc.sync.dma_start(
                        out_d.ap()[qt_i * P : (qt_i + 1) * P, :], osb[:]
                    )

                for qt_i in range(4):
                    w2_block(qt_i)
                for qt_i in range(4, NQ):
                    w2_block(qt_i)

    nc.compile()
    return nc


def _prep_in_maps(inputs):
    import ml_dtypes

    fp8 = ml_dtypes.float8_e4m3

    x = np.asarray(inputs["x"], dtype=np.float32)
    wq = np.ascontiguousarray(
        (np.asarray(inputs["wq"], np.float32) * 32.0).astype(fp8)
    )
    wk = np.ascontiguousarray(
        (np.asarray(inputs["wk"], np.float32) * 32.0).astype(fp8)
    )
    wv = np.ascontiguousarray(
        (np.asarray(inputs["wv"], np.float32) * 32.0).astype(fp8)
    )
    wo = np.ascontiguousarray(
        (np.asarray(inputs["wo"], np.float32) * 64.0).astype(fp8)
    )
    w1 = np.ascontiguousarray(
        np.asarray(inputs["w1"], np.float32).astype(ml_dtypes.bfloat16)
    )
    w2 = np.ascontiguousarray(
        np.asarray(inputs["w2"], np.float32).astype(ml_dtypes.bfloat16)
    )
    b1t = np.ascontiguousarray(
        np.asarray(inputs["b1"], np.float32).reshape(NF, P).T
    )
    b2r = np.ascontiguousarray(
        np.asarray(inputs["b2"], np.float32).reshape(1, D).astype(ml_dtypes.bfloat16)
    )
    ln1a = np.ascontiguousarray(
        np.broadcast_to(np.asarray(inputs["ln1_alpha"], np.float32), (P, D))
    )
    ln1b = np.ascontiguousarray(
        np.broadcast_to(np.asarray(inputs["ln1_bias"], np.float32), (P, D))
    )
    ln2a = np.ascontiguousarray(
        np.broadcast_to(np.asarray(inputs["ln2_alpha"], np.float32), (P, D))
    )
    ln2b = np.ascontiguousarray(
        np.broadcast_to(np.asarray(inputs["ln2_bias"], np.float32), (P, D))
    )
    shared = dict(
        wq=wq, wk=wk, wv=wv, wo=wo, w1=w1, w2=w2,
        b1t=b1t, b2r=b2r, ln1a=ln1a, ln1b=ln1b, ln2a=ln2a, ln2b=ln2b,
    )
    in_maps = []
    for c in range(NCORES):
        b, half = c // 2, c % 2
        xb = x[b]  # [S, D]
        rolled = np.concatenate([xb[half * QR :], xb[: half * QR]], axis=0)
        m = dict(shared)
        m["xt"] = np.ascontiguousarray(rolled.T.astype(fp8))
        m["xh"] = np.ascontiguousarray(
            xb[half * QR : half * QR + QR].astype(ml_dtypes.bfloat16)
        )
        in_maps.append(m)
    return in_maps


def _skip_affine(inputs):
    return (
        np.all(np.asarray(inputs["ln1_alpha"]) == 1.0)
        and np.all(np.asarray(inputs["ln2_alpha"]) == 1.0)
        and np.all(np.asarray(inputs["ln1_bias"]) == 0.0)
        and np.all(np.asarray(inputs["ln2_bias"]) == 0.0)
    )


def kernel(**inputs):
    from concourse.bass_utils import run_bass_kernel_spmd

    sa = bool(_skip_affine(inputs))
    bz = bool(np.all(np.asarray(inputs["b2"]) == 0.0))
    key = ("nc", sa, bz)
    if key not in _CACHE:
        _CACHE[key] = _build(sa, bz)
    nc = _CACHE[key]
    in_maps = _prep_in_maps(inputs)
    res = run_bass_kernel_spmd(nc, in_maps, core_ids=list(range(NCORES)))
    out = np.empty((B, S, D), dtype=np.float32)
    for c in range(NCORES):
        b, half = c // 2, c % 2
        out[b, half * QR : half * QR + QR, :] = res.results[c]["out"]
    return out


# revision 46
# speedup vs baseline: 1.0757x; 1.0182x over previous
"""Trainium2 Bass kernel for a dense transformer encoder block.

Sharding: pure data-parallel, zero collectives. 8 cores; core c handles
batch b = c//2, query rows half = c%2 (1024 of 2048 seq positions).
Each core receives the full (sequence-rotated) x[b]^T so it can compute
K/V over all 2048 keys locally; queries are always columns 0:1024 of the
rotated x^T (attention is permutation-invariant over the key axis).

v3: query-chunk-outer attention with software-pipelined overlap: during
the second query-chunk's attention pass (which is paced by the Scalar
engine's softmax exps), the PE queue is fed the first chunk's wo
projection + LN1 + transposes and its full w1 FFN layer, one work item
per kt-pair slot.  fp8e4 DoubleRow matmuls (2 k-tiles per instruction)
for the V/Q/K projections and the attention*V matmul; fp8 scores
operands and fp8 ctxT/wo.  All fp8 prescales are powers of two and are
compensated exactly:
  - wq,wk x32 -> qh/kh store 32q/32k in fp8; scores psum = 1024*(q.k);
    exp runs with scale=2^-13 and bias=-ln16 (pt = p/16, max ~42 << 240).
  - wv x32, ones-column of V' = 1.0 -> ctx psum rows = 2*sum(p)v, denom
    row = sum(p)/16; reciprocal-normalize yields ctxT = 32*ctx in fp8.
  - wo x64 in fp8 -> wo psum = 2048*attn_out; one fused DVE op computes
    psum*2^-11 + x residual.
Scores run K=128 against zero-padded kh tiles (no PE tiling-mode
switches, keeps ctx DoubleRow weight-loads hidden).  w1 streams in
double-buffered chunks; w2 is resident only for the tail.
"""

import sys

if "/opt/trn_rl_repo" not in sys.path:
    sys.path.insert(0, "/opt/trn_rl_repo")

import numpy as np

B, S, D, H, DK, DFF = 4, 2048, 768, 12, 64, 3072
NCORES = 8
QR = 1024  # query rows per core
EPS = 1e-6
P = 128
NE = D // P  # 6 e-tiles (contraction over model dim)
NEP = NE // 2  # 3 DoubleRow pairs
NS = S // P  # 16 s-tiles (key positions)
NSP = NS // 2  # 8 DoubleRow kt pairs
NQ = QR // P  # 8 q-tiles
NF = DFF // P  # 24 f-tiles
HW = 80  # padded per-head width in V' (64 d + ones col + pad)
LN16 = 2.772588722239781  # ln(16)

_CACHE = {}


def _build(skip_affine, b2_zero=False):
    from contextlib import ExitStack

    import concourse.bass as bass
    import concourse.tile as tile
    from concourse import bacc, mybir
    from concourse.masks import make_identity

    dt = mybir.dt
    f32 = dt.float32
    bf16 = dt.bfloat16
    fp8 = dt.float8e4
    AF = mybir.ActivationFunctionType
    OP = mybir.AluOpType
    DR = mybir.MatmulPerfMode.DoubleRow

    nc = bacc.Bacc("TRN2", target_bir_lowering=False, debug=False)

    xt_d = nc.dram_tensor("xt", [D, S], fp8, kind="ExternalInput")
    xh_d = nc.dram_tensor("xh", [QR, D], bf16, kind="ExternalInput")
    wq_d = nc.dram_tensor("wq", [D, D], fp8, kind="ExternalInput")  # x32
    wk_d = nc.dram_tensor("wk", [D, D], fp8, kind="ExternalInput")  # x32
    wv_d = nc.dram_tensor("wv", [D, D], fp8, kind="ExternalInput")  # x32
    wo_d = nc.dram_tensor("wo", [D, D], fp8, kind="ExternalInput")  # x64
    w1_d = nc.dram_tensor("w1", [D, DFF], bf16, kind="ExternalInput")
    w2_d = nc.dram_tensor("w2", [DFF, D], bf16, kind="ExternalInput")
    b1_d = nc.dram_tensor("b1t", [P, NF], f32, kind="ExternalInput")  # b1 T'd
    b2_d = nc.dram_tensor("b2r", [1, D], bf16, kind="ExternalInput")
    ln1a_d = nc.dram_tensor("ln1a", [P, D], f32, kind="ExternalInput")  # bcast
    ln1b_d = nc.dram_tensor("ln1b", [P, D], f32, kind="ExternalInput")
    ln2a_d = nc.dram_tensor("ln2a", [P, D], f32, kind="ExternalInput")
    ln2b_d = nc.dram_tensor("ln2b", [P, D], f32, kind="ExternalInput")
    out_d = nc.dram_tensor("out", [QR, D], f32, kind="ExternalOutput")

    def dram3(d_ap, p=P):
        return d_ap.rearrange("(n p) s -> p n s", p=p)

    with tile.TileContext(nc) as tc:
        with ExitStack() as ctx:
            const = ctx.enter_context(tc.tile_pool(name="const", bufs=1))
            ones_bf = const.tile([1, P], bf16)
            nc.gpsimd.memset(ones_bf[:], 1.0)
            ident = const.tile([P, P], f32)
            make_identity(nc, ident[:])
            b1_sb = const.tile([P, NF], f32)
            nc.sync.dma_start(b1_sb[:], b1_d.ap())
            b2_sb = const.tile([1, D], bf16)
            nc.sync.dma_start(b2_sb[:], b2_d.ap())
            expb = const.tile([P, 1], f32)  # exp bias: -ln(16)
            nc.gpsimd.memset(expb[:], -LN16)

            # persistent (whole kernel) left-side pools
            xp = ctx.enter_context(tc.tile_pool(name="xp", bufs=1))
            x1 = xp.tile([P, NQ, D], f32, tag="x1")
            x1t = xp.tile([P, NE, QR], bf16, tag="x1t")
            xhp = ctx.enter_context(tc.tile_pool(name="xhp", bufs=1))
            spL = ctx.enter_context(tc.tile_pool(name="spL", bufs=1))
            lnc = ctx.enter_context(tc.tile_pool(name="lnc", bufs=1))
            if skip_affine:
                l1a = l1b = l2a = l2b = None
            else:
                l1a = lnc.tile([P, D], f32, tag="l1a")
                l1b = lnc.tile([P, D], f32, tag="l1b")
                l2a = lnc.tile([P, D], f32, tag="l2a")
                l2b = lnc.tile([P, D], f32, tag="l2b")
                nc.sync.dma_start(l1a[:], ln1a_d.ap())
                nc.sync.dma_start(l1b[:], ln1b_d.ap())
                nc.sync.dma_start(l2a[:], ln2a_d.ap())
                nc.sync.dma_start(l2b[:], ln2b_d.ap())
            w1p = ctx.enter_context(tc.tile_pool(name="w1p", bufs=2))
            # bufs=2: ht(qc1) relu must not wait on ht(qc0)'s w2 readers,
            # which sit later in the PE queue (would deadlock at bufs=1)
            htp = ctx.enter_context(tc.tile_pool(name="htp", bufs=2))

            # right side: ctx^T (fp8, = 32*ctx) + wo (fp8, = 64*wo)
            ctp = ctx.enter_context(tc.tile_pool(name="ctp", bufs=1, side="right"))
            ctxT = ctp.tile([P, NE, QR], fp8, tag="ctxT")
            wo_sb = ctp.tile([P, NE, D], fp8, tag="wo")

            psX = ctx.enter_context(tc.tile_pool(name="psX", bufs=2, space="PSUM"))

            es_ab = ExitStack()  # attention-lifetime pools
            kqv = es_ab.enter_context(tc.tile_pool(name="kqv", bufs=1))
            vo = kqv.tile([P, NSP, 2, H * HW], fp8, tag="vo")
            vo5 = vo[:, :, :, :].rearrange("p s j (h w) -> p s j h w", w=HW)
            kqr = es_ab.enter_context(tc.tile_pool(name="kqr", bufs=6))
            # bufs=3: lets the exp stream run one kt-pair ahead when fill
            # items delay the ctx matmuls that release pt tiles
            ptp = es_ab.enter_context(tc.tile_pool(name="ptp", bufs=3))
            up = es_ab.enter_context(tc.tile_pool(name="up", bufs=2))
            psS = es_ab.enter_context(tc.tile_pool(name="psS", bufs=2, space="PSUM"))
            psC = es_ab.enter_context(tc.tile_pool(name="psC", bufs=1, space="PSUM"))

            def layer_norm(tin, out_ap, a_bc, b_bc, spool):
                st6 = spool.tile([P, 2, 6], f32, tag="st6")
                nc.vector.bn_stats(st6[:, 0, :], tin[:, 0:384])
                nc.vector.bn_stats(st6[:, 1, :], tin[:, 384:768])
                mv = spool.tile([P, 2], f32, tag="mv")
                nc.vector.bn_aggr(mv[:], st6[:])
                std = spool.tile([P, 1], f32, tag="std")
                nc.scalar.activation(
                    std[:], mv[:, 1:2], AF.Sqrt, scale=float(D) / (D - 1)
                )
                stde = spool.tile([P, 1], f32, tag="stde")
                nc.vector.tensor_scalar_add(stde[:], std[:], EPS)
                rstd = spool.tile([P, 1], f32, tag="rstd")
                nc.vector.reciprocal(rstd[:], stde[:])
                if skip_affine:
                    nc.vector.tensor_scalar(
                        out_ap, tin[:], mv[:, 0:1], rstd[:],
                        op0=OP.subtract, op1=OP.mult,
                    )
                else:
                    yc = spool.tile([P, D], f32, tag="yc")
                    nc.vector.tensor_scalar(
                        yc[:], tin[:], mv[:, 0:1], rstd[:],
                        op0=OP.subtract, op1=OP.mult,
                    )
                    y2 = spool.tile([P, D], f32, tag="y2")
                    nc.vector.tensor_tensor(y2[:], yc[:], a_bc, OP.mult)
                    nc.vector.tensor_tensor(out_ap, y2[:], b_bc, OP.add)

            w1tiles = {}

            def w1_dma(qc, c):
                t = w1p.tile([P, NE, 4 * P], bf16, tag="w1c")
                for et in range(NE):
                    nc.sync.dma_start(
                        t[:, et, :],
                        w1_d.ap()[
                            et * P : (et + 1) * P, c * 4 * P : (c + 1) * 4 * P
                        ],
                    )
                w1tiles[(qc, c)] = t

            ht_t = [None, None]

            def w1_item(qc, f_t):
                c = f_t // 4
                if f_t % 4 == 0 and c + 2 < NF // 4:
                    w1_dma(qc, c + 2)
                if f_t == 0:
                    ht_new = htp.tile([P, NF, 512], bf16, tag="ht")
                    ht_t[qc] = ht_new
                ht = ht_t[qc]
                w1c = w1tiles[(qc, c)]
                ps = psX.tile([P, 512], f32, tag="x")
                for et in range(NE):
                    nc.tensor.matmul(
                        ps[:],
                        w1c[:, et, (f_t % 4) * P : (f_t % 4 + 1) * P],
                        x1t[:, et, qc * 512 : (qc + 1) * 512],
                        start=(et == 0),
                        stop=(et == NE - 1),
                    )
                nc.scalar.activation(
                    ht[:, f_t, :], ps[:], AF.Relu, bias=b1_sb[:, f_t : f_t + 1]
                )

            def c_item_a(qt_i, xh_sb, xo, spool=None):
                # wo projection (fp8: psum = 2048*attn) + residual + LN1
                spool = spool if spool is not None else spL
                tsb = spool.tile([P, D], f32, tag="tsb")
                for dc, cw in ((0, 512), (512, 256)):
                    ps = psX.tile([P, 512], f32, tag="x")
                    for dt_i in range(NE):
                        nc.tensor.matmul(
                            ps[:, 0:cw],
                            ctxT[:, dt_i, qt_i * P : (qt_i + 1) * P],
                            wo_sb[:, dt_i, dc : dc + cw],
                            start=(dt_i == 0),
                            stop=(dt_i == NE - 1),
                        )
                    nc.vector.scalar_tensor_tensor(
                        tsb[:, dc : dc + cw],
                        ps[:, 0:cw],
                        float(2.0 ** -11),
                        xh_sb[:, qt_i - xo, dc : dc + cw],
                        op0=OP.mult,
                        op1=OP.add,
                    )
                layer_norm(tsb[:], x1[:, qt_i, :], l1a, l1b, spool)

            def c_item_b(qt_i):
                # x1^T transposes, emitted a couple of slots after c_item_a
                # so the PE never waits on the LN chain
                for dt_i in range(NE):
                    pst = psX.tile([P, 512], f32, tag="x")
                    nc.tensor.transpose(
                        pst[:, 0:P], x1[:, qt_i, dt_i * P : (dt_i + 1) * P],
                        ident[:],
                    )
                    nc.vector.tensor_copy(
                        x1t[:, dt_i, qt_i * P : (qt_i + 1) * P], pst[:, 0:P]
                    )

            def c_item(qt_i, xh_sb, xo):
                c_item_a(qt_i, xh_sb, xo)
                c_item_b(qt_i)

            qh_t = [None] * (H // 2)
            kh_t = [None] * (H // 2)
            pcs = [None, None]

            def kt_body(hp, qc, ktp, fill=None):
                qh = qh_t[hp]
                kh0, kh1 = kh_t[hp]
                pc0, pc1 = pcs
                pt = ptp.tile([P, 2, 1024], fp8, tag="pt")
                for j in range(2):
                    kt_i = 2 * ktp + j
                    ps = psS.tile([P, 1024], f32, tag="psS")
                    for hh, khp in ((0, kh0), (1, kh1)):
                        nc.tensor.matmul(
                            ps[:, hh * 512 : hh * 512 + 512],
                            khp[:, kt_i * P : (kt_i + 1) * P],
                            qh[:, qc * 512 : (qc + 1) * 512],
                            start=True,
                            stop=True,
                        )
                    # pt = exp(scores)/16; scores psum = 1024*(q.k)
                    nc.scalar.activation(
                        pt[:, j, :], ps[:], AF.Exp,
                        bias=expb[:], scale=float(2.0 ** -13),
                    )
                if fill is not None:
                    # overlap work goes between the scores and the ctx
                    # matmuls: the PE would otherwise idle here waiting on
                    # the exps that the ctx matmuls consume
                    fill()
                for hh, pc in ((0, pc0), (1, pc1)):
                    h = 2 * hp + hh
                    nc.tensor.matmul(
                        pc[:],
                        vo5[:, ktp, :, h, 0 : DK + 1],
                        pt[:, :, hh * 512 : hh * 512 + 512],
                        start=(ktp == 0),
                        stop=(ktp == NSP - 1),
                        perf_mode=DR,
                    )

            def chain(hp, qc):
                # softmax normalize: ctxT = 32*ctx in fp8
                pc0, pc1 = pcs
                for hh, pc in ((0, pc0), (1, pc1)):
                    dcp = up.tile([1, 512], f32, tag="dcp")
                    nc.vector.tensor_copy(dcp[:], pc[DK : DK + 1, :])
                    rcp = up.tile([1, 512], f32, tag="rcp")
                    nc.vector.reciprocal_approx_fast(rcp[:], dcp[:])
                    rb = up.tile([DK, 512], f32, tag="rb")
                    nc.gpsimd.partition_broadcast(rb[:], rcp[:])
                    nc.vector.tensor_tensor(
                        ctxT[
                            hh * DK : hh * DK + DK, hp,
                            qc * 512 : (qc + 1) * 512,
                        ],
                        pc[0:DK, :],
                        rb[:],
                        OP.mult,
                    )

            # =================== attention + overlapped C/FFN ===============
            with (
                tc.tile_pool(name="xtp", bufs=1) as xtp,
                tc.tile_pool(name="wp", bufs=1) as wp,
            ):
                # DMA order = first-use order: the first PE work is
                # q_proj(0)/k_proj(0) (xt + wq/wk), then V chunks (wv)
                xt = xtp.tile([P, NE, S], fp8)
                wv_sb = wp.tile([P, NE, D], fp8, tag="wv")
                wq_sb = wp.tile([P, NE, D], fp8, tag="wq")
                wk_sb = wp.tile([P, NE, D], fp8, tag="wk")
                for et in range(NE):
                    nc.sync.dma_start(
                        xt[:, et, :], xt_d.ap()[et * P : (et + 1) * P, :]
                    )
                for et in range(NE):
                    nc.sync.dma_start(
                        wq_sb[:, et, :], wq_d.ap()[et * P : (et + 1) * P, :]
                    )
                    nc.sync.dma_start(
                        wk_sb[:, et, :], wk_d.ap()[et * P : (et + 1) * P, :]
                    )
                for et in range(NE):
                    nc.sync.dma_start(
                        wv_sb[:, et, :], wv_d.ap()[et * P : (et + 1) * P, :]
                    )

                # ones columns of V' (1.0; with wv x32 and pt=p/16 the
                # normalize yields ctxT = 32*ctx)
                ones192 = xtp.tile([P, NS * H], f32, tag="ones192")
                nc.gpsimd.memset(ones192[:], 1.0)
                nc.vector.tensor_copy(
                    vo5[:, :, :, :, DK : DK + 1],
                    ones192[:].rearrange(
                        "p (s j h o) -> p s j h o", s=NSP, j=2, h=H
                    ),
                )

                def v_chunk(st):
                    for dc, cw in ((0, 512), (512, 256)):
                        ps = psX.tile([P, 512], f32, tag="x")
                        for ep in range(NEP):
                            nc.tensor.matmul(
                                ps[:, 0:cw],
                                xt[:, 2 * ep : 2 * ep + 2, st * P : (st + 1) * P],
                                wv_sb[:, 2 * ep : 2 * ep + 2, dc : dc + cw],
                                start=(ep == 0),
                                stop=(ep == NEP - 1),
                                perf_mode=DR,
                            )
                        h0, nh = dc // DK, cw // DK
                        nc.vector.tensor_copy(
                            vo5[:, st // 2, st % 2, h0 : h0 + nh, 0:DK],
                            ps[:, 0:cw].rearrange("p (h w) -> p h w", w=DK),
                        )

                def q_proj(hp):
                    qh = kqr.tile([P, QR], fp8, tag="qh")
                    qh_t[hp] = qh
                    for qc in range(QR // 512):
                        ps = psX.tile([P, 512], f32, tag="x")
                        for ep in range(NEP):
                            nc.tensor.matmul(
                                ps[:],
                                wq_sb[:, 2 * ep : 2 * ep + 2, hp * P : (hp + 1) * P],
                                xt[:, 2 * ep : 2 * ep + 2, qc * 512 : (qc + 1) * 512],
                                start=(ep == 0),
                                stop=(ep == NEP - 1),
                                perf_mode=DR,
                            )
                        nc.vector.tensor_copy(qh[:, qc * 512 : (qc + 1) * 512], ps[:])

                def k_proj(hp):
                    kh0 = kqr.tile([P, S], fp8, tag="kh0")
                    kh1 = kqr.tile([P, S], fp8, tag="kh1")
                    kh_t[hp] = (kh0, kh1)
                    nc.gpsimd.memset(kh0[DK:P, :], 0.0)
                    nc.gpsimd.memset(kh1[0:DK, :], 0.0)
                    for sc in range(S // 512):
                        ps = psX.tile([P, 512], f32, tag="x")
                        for ep in range(NEP):
                            nc.tensor.matmul(
                                ps[:],
                                wk_sb[:, 2 * ep : 2 * ep + 2, hp * P : (hp + 1) * P],
                                xt[:, 2 * ep : 2 * ep + 2, sc * 512 : (sc + 1) * 512],
                                start=(ep == 0),
                                stop=(ep == NEP - 1),
                                perf_mode=DR,
                            )
                        nc.vector.tensor_copy(
                            kh0[0:DK, sc * 512 : (sc + 1) * 512], ps[0:DK, :]
                        )
                        nc.vector.tensor_copy(
                            kh1[DK:P, sc * 512 : (sc + 1) * 512], ps[DK:P, :]
                        )

                q_proj(0)
                k_proj(0)
                for et in range(NE):
                    nc.sync.dma_start(
                        wo_sb[:, et, :], wo_d.ap()[et * P : (et + 1) * P, :]
                    )

                # ------------- qc = 0 pass (V-proj + Q/K fillers) ----------
                for hp in range(H // 2):
                    pc0 = psC.tile([DK + 1, 512], f32, tag="c0")
                    pc1 = psC.tile([DK + 1, 512], f32, tag="c1")
                    pcs[0], pcs[1] = pc0, pc1
                    for ktp in range(NSP):
                        if hp == 0:
                            v_chunk(2 * ktp)
                            v_chunk(2 * ktp + 1)
                        kt_body(hp, 0, ktp)
                    if hp + 1 < H // 2:
                        q_proj(hp + 1)
                        k_proj(hp + 1)
                    chain(hp, 0)
            # xt / wq / wk / wv freed here

            # residual rows for the first half + w1 chunk prefetch
            xh_sb0 = xhp.tile([P, 4, D], bf16, tag="xh")
            for qt_i in range(4):
                nc.sync.dma_start(
                    xh_sb0[:, qt_i, :], xh_d.ap()[qt_i * P : (qt_i + 1) * P, :]
                )
            w1_dma(0, 0)
            w1_dma(0, 1)

            # work items hidden inside the qc=1 attention pass; transposes
            # trail their LN by two slots to hide the DVE chain latency
            items = (
                [("ca", 0), ("ca", 1), ("cb", 0), ("ca", 2), ("cb", 1),
                 ("ca", 3), ("cb", 2), ("cb", 3)]
                + [("w1", f) for f in range(NF)]
            )
            it = [0]

            def emit_item():
                if it[0] >= len(items):
                    return
                kind, a = items[it[0]]
                it[0] += 1
                if kind == "ca":
                    c_item_a(a, xh_sb0, 0)
                elif kind == "cb":
                    c_item_b(a)
                elif kind == "w1":
                    w1_item(0, a)

            # ------------------------- qc = 1 pass ---------------------------
            for hp in range(H // 2):
                pc0 = psC.tile([DK + 1, 512], f32, tag="c0")
                pc1 = psC.tile([DK + 1, 512], f32, tag="c1")
                pcs[0], pcs[1] = pc0, pc1
                for ktp in range(NSP):
                    kt_body(hp, 1, ktp, fill=emit_item)
                chain(hp, 1)
            while it[0] < len(items):
                emit_item()
            # prefetch the tail's residual rows before the phase boundary
            xh_sb1 = xhp.tile([P, 4, D], bf16, tag="xh")
            for qt_i in range(4):
                nc.sync.dma_start(
                    xh_sb1[:, qt_i, :],
                    xh_d.ap()[(qt_i + 4) * P : (qt_i + 5) * P, :],
                )
            es_ab.close()  # free vo / qh / kh / pt / scores+ctx psum

            # ------------------------------ tail -----------------------------
            with (
                tc.tile_pool(name="w2p", bufs=1) as w2p,
                tc.tile_pool(name="sp2", bufs=3) as sp2,
                tc.tile_pool(name="psF", bufs=4, space="PSUM") as psF,
            ):
                w2_sb = w2p.tile([P, NF, D], bf16)
                for fc in range(NF // 4):
                    nc.sync.dma_start(
                        w2_sb[:, fc * 4 : (fc + 1) * 4, :],
                        dram3(w2_d.ap()[fc * 4 * P : (fc + 1) * 4 * P, :]),
                    )
                w1_dma(1, 0)
                w1_dma(1, 1)
                # LN chains use the tail pool (bufs=3) and the transposes
                # trail their LN by one item so the PE never waits on DVE
                c_item_a(4, xh_sb1, 4, sp2)
                c_item_a(5, xh_sb1, 4, sp2)
                c_item_b(4)
                c_item_a(6, xh_sb1, 4, sp2)
                c_item_b(5)
                c_item_a(7, xh_sb1, 4, sp2)
                c_item_b(6)
                c_item_b(7)
                for f_t in range(NF):
                    w1_item(1, f_t)

                def w2_block(qt_i):
                    qc = qt_i // 4
                    ht = ht_t[qc]
                    t2 = sp2.tile([P, D], f32, tag="t2")
                    for dc, cw in ((0, 512), (512, 256)):
                        ps = psF.tile([P, 512], f32, tag="psF")
                        for f_t in range(NF):
                            nc.tensor.matmul(
                                ps[:, 0:cw],
                                ht[:, f_t, (qt_i % 4) * P : (qt_i % 4 + 1) * P],
                                w2_sb[:, f_t, dc : dc + cw],
                                start=(f_t == 0),
                                stop=(b2_zero and f_t == NF - 1),
                            )
                        if not b2_zero:
                            nc.tensor.matmul(
                                ps[:, 0:cw],
                                ones_bf[0:1, 0:P],
                                b2_sb[0:1, dc : dc + cw],
                                start=False,
                                stop=True,
                            )
                        nc.vector.tensor_add(
                            t2[:, dc : dc + cw], x1[:, qt_i, dc : dc + cw],
                            ps[:, 0:cw],
                        )
                    osb = sp2.tile([P, D], f32, tag="osb")
                    layer_norm(t2[:], osb[:], l2a, l2b, sp2)
                    nc.sync.dma_start(
                        out_d.ap()[qt_i * P : (qt_i + 1) * P, :], osb[:]
                    )

                for qt_i in range(4):
                    w2_block(qt_i)
                for qt_i in range(4, NQ):
                    w2_block(qt_i)

    nc.compile()
    return nc


def _prep_in_maps(inputs):
    import ml_dtypes

    fp8 = ml_dtypes.float8_e4m3

    x = np.asarray(inputs["x"], dtype=np.float32)
    wq = np.ascontiguousarray(
        (np.asarray(inputs["wq"], np.float32) * 32.0).astype(fp8)
    )
    wk = np.ascontiguousarray(
        (np.asarray(inputs["wk"], np.float32) * 32.0).astype(fp8)
    )
    wv = np.ascontiguousarray(
        (np.asarray(inputs["wv"], np.float32) * 32.0).astype(fp8)
    )
    wo = np.ascontiguousarray(
        (np.asarray(inputs["wo"], np.float32) * 64.0).astype(fp8)
    )
    w1 = np.ascontiguousarray(
        np.asarray(inputs["w1"], np.float32).astype(ml_dtypes.bfloat16)
    )
    w2 = np.ascontiguousarray(
        np.asarray(inputs["w2"], np.float32).astype(ml_dtypes.bfloat16)
    )
    b1t = np.ascontiguousarray(
        np.asarray(inputs["b1"], np.float32).reshape(NF, P).T
    )
    b2r = np.ascontiguousarray(
        np.asarray(inputs["b2"], np.float32).reshape(1, D).astype(ml_dtypes.bfloat16)
    )
    ln1a = np.ascontiguousarray(
        np.broadcast_to(np.asarray(inputs["ln1_alpha"], np.float32), (P, D))
    )
    ln1b = np.ascontiguousarray(
        np.broadcast_to(np.asarray(inputs["ln1_bias"], np.float32), (P, D))
    )
    ln2a = np.ascontiguousarray(
        np.broadcast_to(np.asarray(inputs["ln2_alpha"], np.float32), (P, D))
    )
    ln2b = np.ascontiguousarray(
        np.broadcast_to(np.asarray(inputs["ln2_bias"], np.float32), (P, D))
    )
    shared = dict(
        wq=wq, wk=wk, wv=wv, wo=wo, w1=w1, w2=w2,
        b1t=b1t, b2r=b2r, ln1a=ln1a, ln1b=ln1b, ln2a=ln2a, ln2b=ln2b,
    )
    in_maps = []
    for c in range(NCORES):
        b, half = c // 2, c % 2
        xb = x[b]  # [S, D]
        rolled = np.concatenate([xb[half * QR :], xb[: half * QR]], axis=0)
        m = dict(shared)
        m["xt"] = np.ascontiguousarray(rolled.T.astype(fp8))
        m["xh"] = np.ascontiguousarray(
            xb[half * QR : half * QR + QR].astype(ml_dtypes.bfloat16)
        )
        in_maps.append(m)
    return in_maps


def _skip_affine(inputs):
    return (
        np.all(np.asarray(inputs["ln1_alpha"]) == 1.0)
        and np.all(np.asarray(inputs["ln2_alpha"]) == 1.0)
        and np.all(np.asarray(inputs["ln1_bias"]) == 0.0)
        and np.all(np.asarray(inputs["ln2_bias"]) == 0.0)
    )


def kernel(**inputs):
    from concourse.bass_utils import run_bass_kernel_spmd

    sa = bool(_skip_affine(inputs))
    bz = bool(np.all(np.asarray(inputs["b2"]) == 0.0))
    key = ("nc", sa, bz)
    if key not in _CACHE:
        _CACHE[key] = _build(sa, bz)
    nc = _CACHE[key]
    in_maps = _prep_in_maps(inputs)
    res = run_bass_kernel_spmd(nc, in_maps, core_ids=list(range(NCORES)))
    out = np.empty((B, S, D), dtype=np.float32)
    for c in range(NCORES):
        b, half = c // 2, c % 2
        out[b, half * QR : half * QR + QR, :] = res.results[c]["out"]
    return out
